# revision 12
# baseline (speedup 1.0000x reference)
import os

import numpy as np

import concourse.bass as bass
import concourse.tile as tile
from concourse import bacc, mybir
from concourse.bass_utils import run_bass_kernel_spmd

N_CORES = 8
R_GRP, F_GRP = 4, 2
B, S, K = 4, 2048, 4096
N_OUT = 4096
M_ALL = B * S
M_SH = M_ALL // R_GRP
N_SH = N_OUT // F_GRP
WRED = N_OUT // N_CORES
KO = K // 128
M_CHUNK = 512
N_MC = M_SH // M_CHUNK
N_NB = N_SH // 128

N_PAIR = 4
KF = 2 * N_PAIR

C_ABS = float(np.float32(0.79788456))
THRESH = 2.0 / 3.0
F32 = mybir.dt.float32
BF16 = mybir.dt.bfloat16
FP8 = mybir.dt.float8e4

_CACHE = {}
LAST_RESULTS = None


def _build_scale():
    nc = bacc.Bacc(None, target_bir_lowering=False, num_devices=N_CORES)
    wred_d = nc.dram_tensor("wredN", [WRED, K], F32, kind="ExternalInput")
    part_d = nc.dram_tensor("partial", [1, 1], F32, kind="ExternalOutput")

    with tile.TileContext(nc) as tc:
        with (
            tc.tile_pool(name="misc", bufs=1) as misc,
            tc.tile_pool(name="redstage", bufs=4) as redstage,
            tc.tile_pool(name="psum_s", bufs=1, space="PSUM") as psum_s_pool,
        ):
            racc = misc.tile([128, 8], F32)
            for t in range(8):
                wf = redstage.tile([128, K // 2], F32, tag="redstage")
                (nc.sync if t % 2 == 0 else nc.scalar).dma_start(
                    wf[:], wred_d.rearrange("(a p) (b c) -> p a b c", p=128, b=2)
                    [:, t // 2, t % 2, :])
                if t % 2 == 0:
                    nc.vector.tensor_reduce(
                        racc[:, t:t + 1], wf[:],
                        axis=mybir.AxisListType.X, op=mybir.AluOpType.add,
                        apply_absolute_value=True)
                else:
                    nc.scalar.activation(
                        wf[:], wf[:], mybir.ActivationFunctionType.Abs,
                        accum_out=racc[:, t:t + 1])
            rsm = misc.tile([128, 8], F32)
            nc.vector.tensor_scalar(
                rsm[:], racc[:], -float(np.float32((K // 2) * np.float32(C_ABS))),
                None, mybir.AluOpType.add)
            r1 = misc.tile([128, 1], F32)
            nc.vector.tensor_reduce(
                r1[:], rsm[:], axis=mybir.AxisListType.X, op=mybir.AluOpType.add)
            ones_col = misc.tile([128, 1], F32)
            nc.vector.memset(ones_col[:], 1.0)
            ps1 = psum_s_pool.tile([1, 1], F32)
            nc.tensor.matmul(ps1[:], lhsT=r1[:], rhs=ones_col[:])
            sc = misc.tile([1, 1], F32)
            nc.vector.tensor_copy(sc[:], ps1[:])
            nc.sync.dma_start(part_d[:], sc[:])

    nc.compile()
    return nc


def _build_main():
    nc = bacc.Bacc(None, target_bir_lowering=False, num_devices=N_CORES)
    xt_d = nc.dram_tensor("xt_sh", [K, M_SH], F32, kind="ExternalInput")
    wt5_d = nc.dram_tensor("wt5", [N_NB, 128, KO, 128], F32, kind="ExternalInput")
    part_d = nc.dram_tensor("partials", [N_CORES], F32, kind="ExternalInput")
    outT = nc.dram_tensor("outT", [N_SH, M_SH], F32, kind="ExternalOutput")

    with tile.TileContext(nc) as tc:
        with (
            tc.tile_pool(name="misc", bufs=1) as misc,
            tc.tile_pool(name="xwin", bufs=2) as xwin,
            tc.tile_pool(name="xstage", bufs=3) as xstage,
            tc.tile_pool(name="wq", bufs=2) as wq_pool,
            tc.tile_pool(name="yq", bufs=2) as yq_pool,
            tc.tile_pool(name="qt", bufs=1) as qt_pool,
            tc.tile_pool(name="outp", bufs=3) as out_pool,
            tc.tile_pool(name="psum", bufs=7, space="PSUM") as psum_pool,
            tc.tile_pool(name="psum_s", bufs=1, space="PSUM") as psum_s_pool,
        ):
            pt = misc.tile([1, N_CORES], F32)
            nc.sync.dma_start(pt[:], part_d.rearrange("(p o) -> p o", p=1))
            s0 = misc.tile([1, 1], F32)
            nc.vector.tensor_reduce(
                s0[:], pt[:], axis=mybir.AxisListType.X, op=mybir.AluOpType.add)
            ones_row = misc.tile([1, 128], F32)
            nc.vector.memset(ones_row[:], 1.0)
            ps_bc = psum_s_pool.tile([128, 1], F32)
            nc.tensor.matmul(ps_bc[:], lhsT=ones_row[:], rhs=s0[:])
            mean_col = misc.tile([128, 1], F32)
            nc.vector.tensor_scalar(
                mean_col[:], ps_bc[:], 1.0 / (N_OUT * K), C_ABS,
                mybir.AluOpType.mult, mybir.AluOpType.add)
            s_col = misc.tile([128, 1], F32)
            nc.vector.tensor_scalar(
                s_col[:], mean_col[:], 1e-5, 1000.0,
                mybir.AluOpType.max, mybir.AluOpType.min)
            thr_col = misc.tile([128, 1], F32)
            nc.vector.tensor_scalar(
                thr_col[:], s_col[:], THRESH, None, mybir.AluOpType.mult)
            nthr_col = misc.tile([128, 1], F32)
            nc.vector.tensor_scalar(
                nthr_col[:], s_col[:], -THRESH, None, mybir.AluOpType.mult)
            sh_col = misc.tile([128, 1], F32)
            nc.vector.tensor_scalar(
                sh_col[:], s_col[:], 0.5, None, mybir.AluOpType.mult)

            qt8 = [qt_pool.tile([128, KF, 128], FP8, name=f"qt8_{nb}")
                   for nb in range(N_NB)] if KF else None
            qtb = [qt_pool.tile([128, KO - KF, 128], BF16, name=f"qtb_{nb}")
                   for nb in range(N_NB)]

            def quant_half(nb, h):
                wq = wq_pool.tile([128, KO // 2, 128], F32, tag="wq",
                                  name=f"wq{nb}_{h}")
                nc.scalar.dma_start(wq[:], wt5_d[nb, :, 16 * h:16 * (h + 1), :])
                wq_f = wq[:].rearrange("p a b -> p (a b)")
                if h == 0:
                    sp = yq_pool.tile([128, 2048], BF16, tag="yq",
                                      name=f"sp{nb}")
                    nc.scalar.activation(
                        sp[:], wq_f, mybir.ActivationFunctionType.Sign,
                        bias=nthr_col[:])
                    sn = yq_pool.tile([128, 2048], BF16, tag="yq",
                                      name=f"sn{nb}")
                    nc.scalar.activation(
                        sn[:], wq_f, mybir.ActivationFunctionType.Sign,
                        bias=thr_col[:])
                    if KF:
                        nc.vector.tensor_tensor(
                            qt8[nb][:].rearrange("p a b -> p (a b)"),
                            sp[:, :KF * 128], sn[:, :KF * 128],
                            mybir.AluOpType.add)
                    nc.vector.tensor_tensor(
                        qtb[nb][:, :16 - KF, :].rearrange("p a b -> p (a b)"),
                        sp[:, KF * 128:], sn[:, KF * 128:],
                        mybir.AluOpType.add)
                else:
                    mpos = yq_pool.tile([128, 2048], BF16, tag="yq",
                                        name=f"mp{nb}")
                    nc.vector.tensor_scalar(
                        mpos[:], wq_f, thr_col[:], 2.0,
                        mybir.AluOpType.is_gt, mybir.AluOpType.mult)
                    mneg = yq_pool.tile([128, 2048], BF16, tag="yq",
                                        name=f"mn{nb}")
                    nc.vector.tensor_scalar(
                        mneg[:], wq_f, nthr_col[:], 2.0,
                        mybir.AluOpType.is_lt, mybir.AluOpType.mult)
                    nc.vector.tensor_tensor(
                        qtb[nb][:, 16 - KF:, :].rearrange("p a b -> p (a b)"),
                        mpos[:], mneg[:], mybir.AluOpType.subtract)

            def load_x_group(mc, inject=None):
                tiles8, tilesb = {}, {}
                for kb in range(KO):
                    if inject and kb in inject:
                        quant_half(*inject[kb])
                    xs = xstage.tile([128, M_CHUNK], F32, tag="xs")
                    nc.scalar.dma_start(
                        xs[:], xt_d[128 * kb:128 * (kb + 1),
                                    M_CHUNK * mc:M_CHUNK * (mc + 1)])
                    if kb < KF:
                        j = kb // 2
                        if kb % 2 == 0:
                            t8 = xwin.tile([128, 2, M_CHUNK], FP8,
                                           tag=f"x8_{j}", name=f"x8_{j}_{mc}")
                            tiles8[j] = t8
                        dst = tiles8[kb // 2][:, kb % 2, :]
                    else:
                        tb = xwin.tile([128, M_CHUNK], BF16,
                                       tag=f"xb_{kb}", name=f"xb_{kb}_{mc}")
                        tilesb[kb] = tb
                        dst = tb[:]
                    if kb % 2 == 0:
                        nc.scalar.mul(dst, xs[:], sh_col[:])
                    else:
                        nc.vector.tensor_scalar(
                            dst, xs[:], sh_col[:], None, mybir.AluOpType.mult)
                return tiles8, tilesb

            def chain(nb, mc, xg):
                tiles8, tilesb = xg
                ps = psum_pool.tile([128, M_CHUNK], F32, tag="ps",
                                    name=f"ps{nb}_{mc}")
                for j in range(N_PAIR):
                    nc.tensor.matmul(
                        ps[:], lhsT=qt8[nb][:, 2 * j:2 * j + 2, :],
                        rhs=tiles8[j][:],
                        start=(j == 0), stop=False,
                        perf_mode=mybir.MatmulPerfMode.DoubleRow)
                for i, kb in enumerate(range(KF, KO)):
                    nc.tensor.matmul(
                        ps[:], lhsT=qtb[nb][:, i, :], rhs=tilesb[kb][:],
                        start=(KF == 0 and i == 0), stop=(kb == KO - 1))
                ob = out_pool.tile([128, M_CHUNK], F32, tag="outp",
                                   name=f"ob{nb}_{mc}")
                if (nb + mc) % 2 == 0:
                    nc.scalar.copy(ob[:], ps[:])
                else:
                    nc.vector.tensor_copy(ob[:], ps[:])
                nc.sync.dma_start(
                    outT[128 * nb:128 * (nb + 1),
                         M_CHUNK * mc:M_CHUNK * (mc + 1)], ob[:])

            quant_half(0, 0)
            xg0 = load_x_group(0, inject={
                2: (0, 1), 5: (1, 0), 8: (1, 1), 12: (2, 0), 16: (2, 1),
                20: (3, 0), 24: (3, 1), 28: (4, 0)})
            xg1 = load_x_group(1, inject={
                0: (4, 1), 4: (5, 0), 8: (5, 1), 12: (6, 0), 16: (6, 1),
                20: (7, 0), 24: (7, 1), 28: (8, 0)})
            for nb in range(8):
                if nb < 7:
                    quant_half(nb + 8, 1)
                    quant_half(nb + 9, 0)
                chain(nb, 0, xg0)
                chain(nb, 1, xg1)
            quant_half(15, 1)
            for nb in range(8, N_NB):
                chain(nb, 0, xg0)
            xg2 = load_x_group(2)
            for nb in range(8, N_NB):
                chain(nb, 1, xg1)
            xg3 = load_x_group(3)
            for nb in range(N_NB):
                chain(nb, 2, xg2)
            for nb in range(N_NB):
                chain(nb, 3, xg3)

    nc.compile()
    return nc


def kernel(x, weight, bias):
    global LAST_RESULTS
    x = np.asarray(x, dtype=np.float32)
    weight = np.ascontiguousarray(np.asarray(weight, dtype=np.float32))
    bias = np.ascontiguousarray(np.asarray(bias, dtype=np.float32))

    if "nc_scale" not in _CACHE:
        _CACHE["nc_scale"] = _build_scale()
        _CACHE["nc_main"] = _build_main()
    nc_scale, nc_main = _CACHE["nc_scale"], _CACHE["nc_main"]

    trace = bool(int(os.environ.get("KERNEL_TRACE", "0")))
    kw = {"trace": True, "trace_cores": [0]} if trace else {}

    in_a = [{"wredN": weight[WRED * c:WRED * (c + 1)]}
            for c in range(N_CORES)]
    res_a = run_bass_kernel_spmd(nc_scale, in_a, list(range(N_CORES)), **kw)
    partials = np.array(
        [res_a.results[c]["partial"][0, 0] for c in range(N_CORES)],
        dtype=np.float32)

    xr = x.reshape(M_ALL, K)
    in_b = []
    for c in range(N_CORES):
        i, j = c // F_GRP, c % F_GRP
        w_sh = weight[N_SH * j:N_SH * (j + 1)]
        wt5 = np.ascontiguousarray(
            w_sh.reshape(N_NB, 128, KO, 128).transpose(0, 3, 2, 1))
        in_b.append({
            "xt_sh": np.ascontiguousarray(xr[M_SH * i:M_SH * (i + 1)].T),
            "wt5": wt5,
            "partials": partials,
        })
    res_b = run_bass_kernel_spmd(nc_main, in_b, list(range(N_CORES)), **kw)
    LAST_RESULTS = (res_a, res_b)

    out = np.empty((M_ALL, N_OUT), dtype=np.float32)
    for c in range(N_CORES):
        i, j = c // F_GRP, c % F_GRP
        out[M_SH * i:M_SH * (i + 1), N_SH * j:N_SH * (j + 1)] = \
            res_b.results[c]["outT"].T
    if bias.any():
        out += bias[None, :]
    return out.reshape(B, S, N_OUT)


# revision 14
# speedup vs baseline: 1.0288x; 1.0288x over previous
import os

import numpy as np

import concourse.bass as bass
import concourse.tile as tile
from concourse import bacc, mybir
from concourse.bass_utils import run_bass_kernel_spmd

N_CORES = 8
R_GRP, F_GRP = 4, 2
B, S, K = 4, 2048, 4096
N_OUT = 4096
M_ALL = B * S
M_SH = M_ALL // R_GRP
N_SH = N_OUT // F_GRP
WRED = N_OUT // N_CORES
KO = K // 128
M_CHUNK = 512
N_MC = M_SH // M_CHUNK
N_NB = N_SH // 128

N_PAIR = 4
KF = 2 * N_PAIR

C_ABS = float(np.float32(0.79788456))
THRESH = 2.0 / 3.0
F32 = mybir.dt.float32
BF16 = mybir.dt.bfloat16
FP8 = mybir.dt.float8e4

_CACHE = {}
LAST_RESULTS = None


def _build_scale():
    nc = bacc.Bacc(None, target_bir_lowering=False, num_devices=N_CORES)
    wred_d = nc.dram_tensor("wredN", [WRED, K], F32, kind="ExternalInput")
    part_d = nc.dram_tensor("partial", [1, 1], F32, kind="ExternalOutput")

    with tile.TileContext(nc) as tc:
        with (
            tc.tile_pool(name="misc", bufs=1) as misc,
            tc.tile_pool(name="redstage", bufs=4) as redstage,
            tc.tile_pool(name="psum_s", bufs=1, space="PSUM") as psum_s_pool,
        ):
            racc = misc.tile([128, 8], F32)
            for t in range(8):
                wf = redstage.tile([128, K // 2], F32, tag="redstage")
                (nc.sync if t % 2 == 0 else nc.scalar).dma_start(
                    wf[:], wred_d.rearrange("(a p) (b c) -> p a b c", p=128, b=2)
                    [:, t // 2, t % 2, :])
                if t % 2 == 0:
                    nc.vector.tensor_reduce(
                        racc[:, t:t + 1], wf[:],
                        axis=mybir.AxisListType.X, op=mybir.AluOpType.add,
                        apply_absolute_value=True)
                else:
                    nc.scalar.activation(
                        wf[:], wf[:], mybir.ActivationFunctionType.Abs,
                        accum_out=racc[:, t:t + 1])
            rsm = misc.tile([128, 8], F32)
            nc.vector.tensor_scalar(
                rsm[:], racc[:], -float(np.float32((K // 2) * np.float32(C_ABS))),
                None, mybir.AluOpType.add)
            r1 = misc.tile([128, 1], F32)
            nc.vector.tensor_reduce(
                r1[:], rsm[:], axis=mybir.AxisListType.X, op=mybir.AluOpType.add)
            ones_col = misc.tile([128, 1], F32)
            nc.vector.memset(ones_col[:], 1.0)
            ps1 = psum_s_pool.tile([1, 1], F32)
            nc.tensor.matmul(ps1[:], lhsT=r1[:], rhs=ones_col[:])
            sc = misc.tile([1, 1], F32)
            nc.vector.tensor_copy(sc[:], ps1[:])
            nc.sync.dma_start(part_d[:], sc[:])

    nc.compile()
    return nc


def _build_main():
    nc = bacc.Bacc(None, target_bir_lowering=False, num_devices=N_CORES)
    xt_d = nc.dram_tensor("xt_sh", [K, M_SH], F32, kind="ExternalInput")
    wt5_d = nc.dram_tensor("wt5", [N_NB, 128, KO, 128], F32, kind="ExternalInput")
    part_d = nc.dram_tensor("partials", [N_CORES], F32, kind="ExternalInput")
    outT = nc.dram_tensor("outT", [N_SH, M_SH], F32, kind="ExternalOutput")

    with tile.TileContext(nc) as tc:
        with (
            tc.tile_pool(name="misc", bufs=1) as misc,
            tc.tile_pool(name="xwin", bufs=2) as xwin,
            tc.tile_pool(name="xstage", bufs=3) as xstage,
            tc.tile_pool(name="wq", bufs=2) as wq_pool,
            tc.tile_pool(name="yq", bufs=2) as yq_pool,
            tc.tile_pool(name="qt", bufs=1) as qt_pool,
            tc.tile_pool(name="outp", bufs=3) as out_pool,
            tc.tile_pool(name="psum", bufs=7, space="PSUM") as psum_pool,
            tc.tile_pool(name="psum_s", bufs=1, space="PSUM") as psum_s_pool,
        ):
            pt = misc.tile([1, N_CORES], F32)
            nc.sync.dma_start(pt[:], part_d.rearrange("(p o) -> p o", p=1))
            s0 = misc.tile([1, 1], F32)
            nc.vector.tensor_reduce(
                s0[:], pt[:], axis=mybir.AxisListType.X, op=mybir.AluOpType.add)
            ones_row = misc.tile([1, 128], F32)
            nc.vector.memset(ones_row[:], 1.0)
            ps_bc = psum_s_pool.tile([128, 1], F32)
            nc.tensor.matmul(ps_bc[:], lhsT=ones_row[:], rhs=s0[:])
            mean_col = misc.tile([128, 1], F32)
            nc.vector.tensor_scalar(
                mean_col[:], ps_bc[:], 1.0 / (N_OUT * K), C_ABS,
                mybir.AluOpType.mult, mybir.AluOpType.add)
            s_col = misc.tile([128, 1], F32)
            nc.vector.tensor_scalar(
                s_col[:], mean_col[:], 1e-5, 1000.0,
                mybir.AluOpType.max, mybir.AluOpType.min)
            thr_col = misc.tile([128, 1], F32)
            nc.vector.tensor_scalar(
                thr_col[:], s_col[:], THRESH, None, mybir.AluOpType.mult)
            nthr_col = misc.tile([128, 1], F32)
            nc.vector.tensor_scalar(
                nthr_col[:], s_col[:], -THRESH, None, mybir.AluOpType.mult)
            sh_col = misc.tile([128, 1], F32)
            nc.vector.tensor_scalar(
                sh_col[:], s_col[:], 0.5, None, mybir.AluOpType.mult)

            qt8 = [qt_pool.tile([128, KF, 128], FP8, name=f"qt8_{nb}")
                   for nb in range(N_NB)] if KF else None
            qtb = [qt_pool.tile([128, KO - KF, 128], BF16, name=f"qtb_{nb}")
                   for nb in range(N_NB)]

            def quant_half(nb, h):
                wq = wq_pool.tile([128, KO // 2, 128], F32, tag="wq",
                                  name=f"wq{nb}_{h}")
                nc.sync.dma_start(wq[:], wt5_d[nb, :, 16 * h:16 * (h + 1), :])
                wq_f = wq[:].rearrange("p a b -> p (a b)")
                if h == 0:
                    sp = yq_pool.tile([128, 2048], BF16, tag="yq",
                                      name=f"sp{nb}")
                    nc.scalar.activation(
                        sp[:], wq_f, mybir.ActivationFunctionType.Sign,
                        bias=nthr_col[:])
                    sn = yq_pool.tile([128, 2048], BF16, tag="yq",
                                      name=f"sn{nb}")
                    nc.scalar.activation(
                        sn[:], wq_f, mybir.ActivationFunctionType.Sign,
                        bias=thr_col[:])
                    if KF:
                        nc.vector.tensor_tensor(
                            qt8[nb][:].rearrange("p a b -> p (a b)"),
                            sp[:, :KF * 128], sn[:, :KF * 128],
                            mybir.AluOpType.add)
                    if KF < 16:
                        nc.vector.tensor_tensor(
                            qtb[nb][:, :16 - KF, :].rearrange("p a b -> p (a b)"),
                            sp[:, KF * 128:], sn[:, KF * 128:],
                            mybir.AluOpType.add)
                else:
                    mpos = yq_pool.tile([128, 2048], BF16, tag="yq",
                                        name=f"mp{nb}")
                    nc.vector.tensor_scalar(
                        mpos[:], wq_f, thr_col[:], 2.0,
                        mybir.AluOpType.is_gt, mybir.AluOpType.mult)
                    mneg = yq_pool.tile([128, 2048], BF16, tag="yq",
                                        name=f"mn{nb}")
                    nc.vector.tensor_scalar(
                        mneg[:], wq_f, nthr_col[:], 2.0,
                        mybir.AluOpType.is_lt, mybir.AluOpType.mult)
                    nc.vector.tensor_tensor(
                        qtb[nb][:, 16 - KF:, :].rearrange("p a b -> p (a b)"),
                        mpos[:], mneg[:], mybir.AluOpType.subtract)

            def load_x_group(mc, inject=None):
                tiles8, tilesb = {}, {}
                for kb in range(KO):
                    if inject and kb in inject:
                        quant_half(*inject[kb])
                    xs = xstage.tile([128, M_CHUNK], F32, tag="xs")
                    nc.sync.dma_start(
                        xs[:], xt_d[128 * kb:128 * (kb + 1),
                                    M_CHUNK * mc:M_CHUNK * (mc + 1)])
                    if kb < KF:
                        j = kb // 2
                        if kb % 2 == 0:
                            t8 = xwin.tile([128, 2, M_CHUNK], FP8,
                                           tag=f"x8_{j}", name=f"x8_{j}_{mc}")
                            tiles8[j] = t8
                        dst = tiles8[kb // 2][:, kb % 2, :]
                    else:
                        tb = xwin.tile([128, M_CHUNK], BF16,
                                       tag=f"xb_{kb}", name=f"xb_{kb}_{mc}")
                        tilesb[kb] = tb
                        dst = tb[:]
                    if kb % 2 == 0:
                        nc.scalar.mul(dst, xs[:], sh_col[:])
                    else:
                        nc.vector.tensor_scalar(
                            dst, xs[:], sh_col[:], None, mybir.AluOpType.mult)
                return tiles8, tilesb

            def chain(nb, mc, xg):
                tiles8, tilesb = xg
                ps = psum_pool.tile([128, M_CHUNK], F32, tag="ps",
                                    name=f"ps{nb}_{mc}")
                for j in range(N_PAIR):
                    nc.tensor.matmul(
                        ps[:], lhsT=qt8[nb][:, 2 * j:2 * j + 2, :],
                        rhs=tiles8[j][:],
                        start=(j == 0), stop=False,
                        perf_mode=mybir.MatmulPerfMode.DoubleRow)
                for i, kb in enumerate(range(KF, KO)):
                    nc.tensor.matmul(
                        ps[:], lhsT=qtb[nb][:, i, :], rhs=tilesb[kb][:],
                        start=(KF == 0 and i == 0), stop=(kb == KO - 1))
                ob = out_pool.tile([128, M_CHUNK], F32, tag="outp",
                                   name=f"ob{nb}_{mc}")
                if (nb + mc) % 2 == 0:
                    nc.scalar.copy(ob[:], ps[:])
                else:
                    nc.vector.tensor_copy(ob[:], ps[:])
                nc.scalar.dma_start(
                    outT[128 * nb:128 * (nb + 1),
                         M_CHUNK * mc:M_CHUNK * (mc + 1)], ob[:])

            quant_half(0, 0)
            xg0 = load_x_group(0, inject={
                2: (0, 1), 5: (1, 0), 8: (1, 1), 12: (2, 0), 16: (2, 1),
                20: (3, 0), 24: (3, 1), 28: (4, 0)})
            xg1 = load_x_group(1, inject={
                0: (4, 1), 4: (5, 0), 8: (5, 1), 12: (6, 0), 16: (6, 1),
                20: (7, 0), 24: (7, 1), 28: (8, 0)})
            for nb in range(8):
                if nb < 7:
                    quant_half(nb + 8, 1)
                    quant_half(nb + 9, 0)
                chain(nb, 0, xg0)
                chain(nb, 1, xg1)
            quant_half(15, 1)
            for nb in range(8, N_NB):
                chain(nb, 0, xg0)
            xg2 = load_x_group(2)
            for nb in range(8, N_NB):
                chain(nb, 1, xg1)
            xg3 = load_x_group(3)
            for nb in range(N_NB):
                chain(nb, 2, xg2)
            for nb in range(N_NB):
                chain(nb, 3, xg3)

    nc.compile()
    return nc


def kernel(x, weight, bias):
    global LAST_RESULTS
    x = np.asarray(x, dtype=np.float32)
    weight = np.ascontiguousarray(np.asarray(weight, dtype=np.float32))
    bias = np.ascontiguousarray(np.asarray(bias, dtype=np.float32))

    if "nc_scale" not in _CACHE:
        _CACHE["nc_scale"] = _build_scale()
        _CACHE["nc_main"] = _build_main()
    nc_scale, nc_main = _CACHE["nc_scale"], _CACHE["nc_main"]

    trace = bool(int(os.environ.get("KERNEL_TRACE", "0")))
    kw = {"trace": True, "trace_cores": [0]} if trace else {}

    in_a = [{"wredN": weight[WRED * c:WRED * (c + 1)]}
            for c in range(N_CORES)]
    res_a = run_bass_kernel_spmd(nc_scale, in_a, list(range(N_CORES)), **kw)
    partials = np.array(
        [res_a.results[c]["partial"][0, 0] for c in range(N_CORES)],
        dtype=np.float32)

    xr = x.reshape(M_ALL, K)
    in_b = []
    for c in range(N_CORES):
        i, j = c // F_GRP, c % F_GRP
        w_sh = weight[N_SH * j:N_SH * (j + 1)]
        wt5 = np.ascontiguousarray(
            w_sh.reshape(N_NB, 128, KO, 128).transpose(0, 3, 2, 1))
        in_b.append({
            "xt_sh": np.ascontiguousarray(xr[M_SH * i:M_SH * (i + 1)].T),
            "wt5": wt5,
            "partials": partials,
        })
    res_b = run_bass_kernel_spmd(nc_main, in_b, list(range(N_CORES)), **kw)
    LAST_RESULTS = (res_a, res_b)

    out = np.empty((M_ALL, N_OUT), dtype=np.float32)
    for c in range(N_CORES):
        i, j = c // F_GRP, c % F_GRP
        out[M_SH * i:M_SH * (i + 1), N_SH * j:N_SH * (j + 1)] = \
            res_b.results[c]["outT"].T
    if bias.any():
        out += bias[None, :]
    return out.reshape(B, S, N_OUT)


# revision 16
# speedup vs baseline: 1.1436x; 1.1116x over previous
import os

import numpy as np

import concourse.bass as bass
import concourse.tile as tile
from concourse import bacc, mybir
from concourse.bass_utils import run_bass_kernel_spmd

N_CORES = 8
R_GRP, F_GRP = 4, 2
B, S, K = 4, 2048, 4096
N_OUT = 4096
M_ALL = B * S
M_SH = M_ALL // R_GRP
N_SH = N_OUT // F_GRP
WRED = N_OUT // N_CORES
KO = K // 128
M_CHUNK = 512
N_MC = M_SH // M_CHUNK
N_NB = N_SH // 128

N_PAIR = 4
KF = 2 * N_PAIR

C_ABS = float(np.float32(0.79788456))
THRESH = 2.0 / 3.0
F32 = mybir.dt.float32
BF16 = mybir.dt.bfloat16
FP8 = mybir.dt.float8e4

_CACHE = {}
LAST_RESULTS = None


def _build_scale():
    nc = bacc.Bacc(None, target_bir_lowering=False, num_devices=N_CORES)
    wred_d = nc.dram_tensor("wredN", [WRED, K], F32, kind="ExternalInput")
    part_d = nc.dram_tensor("partial", [1, 1], F32, kind="ExternalOutput")

    with tile.TileContext(nc) as tc:
        with (
            tc.tile_pool(name="misc", bufs=1) as misc,
            tc.tile_pool(name="redstage", bufs=4) as redstage,
            tc.tile_pool(name="psum_s", bufs=1, space="PSUM") as psum_s_pool,
        ):
            racc = misc.tile([128, 8], F32)
            for t in range(8):
                wf = redstage.tile([128, K // 2], F32, tag="redstage")
                (nc.sync if t % 2 == 0 else nc.scalar).dma_start(
                    wf[:], wred_d.rearrange("(a p) (b c) -> p a b c", p=128, b=2)
                    [:, t // 2, t % 2, :])
                if t % 2 == 0:
                    nc.vector.tensor_reduce(
                        racc[:, t:t + 1], wf[:],
                        axis=mybir.AxisListType.X, op=mybir.AluOpType.add,
                        apply_absolute_value=True)
                else:
                    nc.scalar.activation(
                        wf[:], wf[:], mybir.ActivationFunctionType.Abs,
                        accum_out=racc[:, t:t + 1])
            rsm = misc.tile([128, 8], F32)
            nc.vector.tensor_scalar(
                rsm[:], racc[:], -float(np.float32((K // 2) * np.float32(C_ABS))),
                None, mybir.AluOpType.add)
            r1 = misc.tile([128, 1], F32)
            nc.vector.tensor_reduce(
                r1[:], rsm[:], axis=mybir.AxisListType.X, op=mybir.AluOpType.add)
            ones_col = misc.tile([128, 1], F32)
            nc.vector.memset(ones_col[:], 1.0)
            ps1 = psum_s_pool.tile([1, 1], F32)
            nc.tensor.matmul(ps1[:], lhsT=r1[:], rhs=ones_col[:])
            sc = misc.tile([1, 1], F32)
            nc.vector.tensor_copy(sc[:], ps1[:])
            nc.sync.dma_start(part_d[:], sc[:])

    nc.compile()
    return nc


def _build_main():
    nc = bacc.Bacc(None, target_bir_lowering=False, num_devices=N_CORES)
    xt_d = nc.dram_tensor("xt_sh", [K, M_SH], F32, kind="ExternalInput")
    wt5_d = nc.dram_tensor("wt5", [N_NB, 128, KO, 128], F32, kind="ExternalInput")
    part_d = nc.dram_tensor("partials", [N_CORES], F32, kind="ExternalInput")
    outT = nc.dram_tensor("outT", [N_SH, M_SH], F32, kind="ExternalOutput")

    with tile.TileContext(nc) as tc:
        with (
            tc.tile_pool(name="misc", bufs=1) as misc,
            tc.tile_pool(name="xwin", bufs=2) as xwin,
            tc.tile_pool(name="xstage", bufs=3) as xstage,
            tc.tile_pool(name="wq", bufs=2) as wq_pool,
            tc.tile_pool(name="yq", bufs=2) as yq_pool,
            tc.tile_pool(name="qt", bufs=1) as qt_pool,
            tc.tile_pool(name="outp", bufs=3) as out_pool,
            tc.tile_pool(name="psum", bufs=7, space="PSUM") as psum_pool,
            tc.tile_pool(name="psum_s", bufs=1, space="PSUM") as psum_s_pool,
        ):
            pt = misc.tile([1, N_CORES], F32)
            nc.sync.dma_start(pt[:], part_d.rearrange("(p o) -> p o", p=1))
            s0 = misc.tile([1, 1], F32)
            nc.vector.tensor_reduce(
                s0[:], pt[:], axis=mybir.AxisListType.X, op=mybir.AluOpType.add)
            ones_row = misc.tile([1, 128], F32)
            nc.vector.memset(ones_row[:], 1.0)
            ps_bc = psum_s_pool.tile([128, 1], F32)
            nc.tensor.matmul(ps_bc[:], lhsT=ones_row[:], rhs=s0[:])
            mean_col = misc.tile([128, 1], F32)
            nc.vector.tensor_scalar(
                mean_col[:], ps_bc[:], 1.0 / (N_OUT * K), C_ABS,
                mybir.AluOpType.mult, mybir.AluOpType.add)
            s_col = misc.tile([128, 1], F32)
            nc.vector.tensor_scalar(
                s_col[:], mean_col[:], 1e-5, 1000.0,
                mybir.AluOpType.max, mybir.AluOpType.min)
            thr_col = misc.tile([128, 1], F32)
            nc.vector.tensor_scalar(
                thr_col[:], s_col[:], THRESH, None, mybir.AluOpType.mult)
            nthr_col = misc.tile([128, 1], F32)
            nc.vector.tensor_scalar(
                nthr_col[:], s_col[:], -THRESH, None, mybir.AluOpType.mult)
            sh_col = misc.tile([128, 1], F32)
            nc.vector.tensor_scalar(
                sh_col[:], s_col[:], 0.5, None, mybir.AluOpType.mult)

            qt8 = [qt_pool.tile([128, KF, 128], FP8, name=f"qt8_{nb}")
                   for nb in range(N_NB)] if KF else None
            qtb = [qt_pool.tile([128, KO - KF, 128], BF16, name=f"qtb_{nb}")
                   for nb in range(N_NB)]

            def quant_half(nb, h):
                wq = wq_pool.tile([128, KO // 2, 128], F32, tag="wq",
                                  name=f"wq{nb}_{h}")
                nc.sync.dma_start(wq[:], wt5_d[nb, :, 16 * h:16 * (h + 1), :])
                wq_f = wq[:].rearrange("p a b -> p (a b)")
                if h == 0:
                    sp = yq_pool.tile([128, 2048], BF16, tag="yq",
                                      name=f"sp{nb}")
                    nc.scalar.activation(
                        sp[:], wq_f, mybir.ActivationFunctionType.Sign,
                        bias=nthr_col[:])
                    sn = yq_pool.tile([128, 2048], BF16, tag="yq",
                                      name=f"sn{nb}")
                    nc.scalar.activation(
                        sn[:], wq_f, mybir.ActivationFunctionType.Sign,
                        bias=thr_col[:])
                    if KF:
                        nc.vector.tensor_tensor(
                            qt8[nb][:].rearrange("p a b -> p (a b)"),
                            sp[:, :KF * 128], sn[:, :KF * 128],
                            mybir.AluOpType.add)
                    if KF < 16:
                        nc.vector.tensor_tensor(
                            qtb[nb][:, :16 - KF, :].rearrange("p a b -> p (a b)"),
                            sp[:, KF * 128:], sn[:, KF * 128:],
                            mybir.AluOpType.add)
                else:
                    mpos = yq_pool.tile([128, 2048], BF16, tag="yq",
                                        name=f"mp{nb}")
                    nc.vector.tensor_scalar(
                        mpos[:], wq_f, thr_col[:], 2.0,
                        mybir.AluOpType.is_gt, mybir.AluOpType.mult)
                    mneg = yq_pool.tile([128, 2048], BF16, tag="yq",
                                        name=f"mn{nb}")
                    nc.vector.tensor_scalar(
                        mneg[:], wq_f, nthr_col[:], 2.0,
                        mybir.AluOpType.is_lt, mybir.AluOpType.mult)
                    nc.vector.tensor_tensor(
                        qtb[nb][:, 16 - KF:, :].rearrange("p a b -> p (a b)"),
                        mpos[:], mneg[:], mybir.AluOpType.subtract)

            def emit_x_chunk(mc, kb, xg):
                tiles8, tilesb = xg
                xs = xstage.tile([128, M_CHUNK], F32, tag="xs")
                nc.sync.dma_start(
                    xs[:], xt_d[128 * kb:128 * (kb + 1),
                                M_CHUNK * mc:M_CHUNK * (mc + 1)])
                if kb < KF:
                    j = kb // 2
                    if kb % 2 == 0:
                        t8 = xwin.tile([128, 2, M_CHUNK], FP8,
                                       tag=f"x8_{j}", name=f"x8_{j}_{mc}")
                        tiles8[j] = t8
                    dst = tiles8[kb // 2][:, kb % 2, :]
                else:
                    tb = xwin.tile([128, M_CHUNK], BF16,
                                   tag=f"xb_{kb}", name=f"xb_{kb}_{mc}")
                    tilesb[kb] = tb
                    dst = tb[:]
                if kb % 2 == 0:
                    nc.scalar.mul(dst, xs[:], sh_col[:])
                else:
                    nc.vector.tensor_scalar(
                        dst, xs[:], sh_col[:], None, mybir.AluOpType.mult)

            def load_x_group(mc, inject=None):
                xg = ({}, {})
                for kb in range(KO):
                    if inject and kb in inject:
                        quant_half(*inject[kb])
                    emit_x_chunk(mc, kb, xg)
                return xg

            def chain(nb, mc, xg):
                tiles8, tilesb = xg
                ps = psum_pool.tile([128, M_CHUNK], F32, tag="ps",
                                    name=f"ps{nb}_{mc}")
                for j in range(N_PAIR):
                    nc.tensor.matmul(
                        ps[:], lhsT=qt8[nb][:, 2 * j:2 * j + 2, :],
                        rhs=tiles8[j][:],
                        start=(j == 0), stop=False,
                        perf_mode=mybir.MatmulPerfMode.DoubleRow)
                for i, kb in enumerate(range(KF, KO)):
                    nc.tensor.matmul(
                        ps[:], lhsT=qtb[nb][:, i, :], rhs=tilesb[kb][:],
                        start=(KF == 0 and i == 0), stop=(kb == KO - 1))
                ob = out_pool.tile([128, M_CHUNK], F32, tag="outp",
                                   name=f"ob{nb}_{mc}")
                nc.scalar.copy(ob[:], ps[:])
                nc.scalar.dma_start(
                    outT[128 * nb:128 * (nb + 1),
                         M_CHUNK * mc:M_CHUNK * (mc + 1)], ob[:])

            quant_half(0, 0)
            quant_half(0, 1)
            xg0 = load_x_group(0, inject={
                4: (1, 0), 10: (1, 1), 16: (2, 0), 22: (2, 1), 28: (3, 0)})
            h_list = [(3, 1)] + [(q, h) for q in range(4, N_NB)
                                 for h in (0, 1)]
            hi = 0
            xg1 = ({}, {})
            for nb in range(N_NB):
                for _ in range(2):
                    if hi < len(h_list):
                        quant_half(*h_list[hi])
                        hi += 1
                emit_x_chunk(1, 2 * nb, xg1)
                emit_x_chunk(1, 2 * nb + 1, xg1)
                chain(nb, 0, xg0)
            xg2 = ({}, {})
            for nb in range(N_NB):
                emit_x_chunk(2, 2 * nb, xg2)
                emit_x_chunk(2, 2 * nb + 1, xg2)
                chain(nb, 1, xg1)
            xg3 = ({}, {})
            for nb in range(N_NB):
                emit_x_chunk(3, 2 * nb, xg3)
                emit_x_chunk(3, 2 * nb + 1, xg3)
                chain(nb, 2, xg2)
            for nb in range(N_NB):
                chain(nb, 3, xg3)

    nc.compile()
    return nc


def _build_main():
    nc = bacc.Bacc(None, target_bir_lowering=False, num_devices=N_CORES)
    xt_d = nc.dram_tensor("xt_sh", [K, M_SH], F32, kind="ExternalInput")
    wt5_d = nc.dram_tensor("wt5", [N_NB, 128, KO, 128], F32, kind="ExternalInput")
    part_d = nc.dram_tensor("partials", [N_CORES], F32, kind="ExternalInput")
    outT = nc.dram_tensor("outT", [N_SH, M_SH], F32, kind="ExternalOutput")

    with tile.TileContext(nc) as tc:
        with (
            tc.tile_pool(name="misc", bufs=1) as misc,
            tc.tile_pool(name="xwin", bufs=2) as xwin,
            tc.tile_pool(name="xstage", bufs=3) as xstage,
            tc.tile_pool(name="wq", bufs=2) as wq_pool,
            tc.tile_pool(name="yq", bufs=2) as yq_pool,
            tc.tile_pool(name="qt", bufs=1) as qt_pool,
            tc.tile_pool(name="outp", bufs=3) as out_pool,
            tc.tile_pool(name="psum", bufs=7, space="PSUM") as psum_pool,
            tc.tile_pool(name="psum_s", bufs=1, space="PSUM") as psum_s_pool,
        ):
            pt = misc.tile([1, N_CORES], F32)
            nc.sync.dma_start(pt[:], part_d.rearrange("(p o) -> p o", p=1))
            s0 = misc.tile([1, 1], F32)
            nc.vector.tensor_reduce(
                s0[:], pt[:], axis=mybir.AxisListType.X, op=mybir.AluOpType.add)
            ones_row = misc.tile([1, 128], F32)
            nc.vector.memset(ones_row[:], 1.0)
            ps_bc = psum_s_pool.tile([128, 1], F32)
            nc.tensor.matmul(ps_bc[:], lhsT=ones_row[:], rhs=s0[:])
            mean_col = misc.tile([128, 1], F32)
            nc.vector.tensor_scalar(
                mean_col[:], ps_bc[:], 1.0 / (N_OUT * K), C_ABS,
                mybir.AluOpType.mult, mybir.AluOpType.add)
            s_col = misc.tile([128, 1], F32)
            nc.vector.tensor_scalar(
                s_col[:], mean_col[:], 1e-5, 1000.0,
                mybir.AluOpType.max, mybir.AluOpType.min)
            thr_col = misc.tile([128, 1], F32)
            nc.vector.tensor_scalar(
                thr_col[:], s_col[:], THRESH, None, mybir.AluOpType.mult)
            nthr_col = misc.tile([128, 1], F32)
            nc.vector.tensor_scalar(
                nthr_col[:], s_col[:], -THRESH, None, mybir.AluOpType.mult)
            sh_col = misc.tile([128, 1], F32)
            nc.vector.tensor_scalar(
                sh_col[:], s_col[:], 0.5, None, mybir.AluOpType.mult)

            qt8 = [qt_pool.tile([128, KF, 128], FP8, name=f"qt8_{nb}")
                   for nb in range(N_NB)] if KF else None
            qtb = [qt_pool.tile([128, KO - KF, 128], BF16, name=f"qtb_{nb}")
                   for nb in range(N_NB)]

            def quant_half(nb, h):
                wq = wq_pool.tile([128, KO // 2, 128], F32, tag="wq",
                                  name=f"wq{nb}_{h}")
                nc.sync.dma_start(wq[:], wt5_d[nb, :, 16 * h:16 * (h + 1), :])
                wq_f = wq[:].rearrange("p a b -> p (a b)")
                if h == 0:
                    sp = yq_pool.tile([128, 2048], BF16, tag="yq",
                                      name=f"sp{nb}")
                    nc.scalar.activation(
                        sp[:], wq_f, mybir.ActivationFunctionType.Sign,
                        bias=nthr_col[:])
                    sn = yq_pool.tile([128, 2048], BF16, tag="yq",
                                      name=f"sn{nb}")
                    nc.scalar.activation(
                        sn[:], wq_f, mybir.ActivationFunctionType.Sign,
                        bias=thr_col[:])
                    if KF:
                        nc.vector.tensor_tensor(
                            qt8[nb][:].rearrange("p a b -> p (a b)"),
                            sp[:, :KF * 128], sn[:, :KF * 128],
                            mybir.AluOpType.add)
                    if KF < 16:
                        nc.vector.tensor_tensor(
                            qtb[nb][:, :16 - KF, :].rearrange("p a b -> p (a b)"),
                            sp[:, KF * 128:], sn[:, KF * 128:],
                            mybir.AluOpType.add)
                else:
                    mpos = yq_pool.tile([128, 2048], BF16, tag="yq",
                                        name=f"mp{nb}")
                    nc.vector.tensor_scalar(
                        mpos[:], wq_f, thr_col[:], 2.0,
                        mybir.AluOpType.is_gt, mybir.AluOpType.mult)
                    mneg = yq_pool.tile([128, 2048], BF16, tag="yq",
                                        name=f"mn{nb}")
                    nc.vector.tensor_scalar(
                        mneg[:], wq_f, nthr_col[:], 2.0,
                        mybir.AluOpType.is_lt, mybir.AluOpType.mult)
                    nc.vector.tensor_tensor(
                        qtb[nb][:, 16 - KF:, :].rearrange("p a b -> p (a b)"),
                        mpos[:], mneg[:], mybir.AluOpType.subtract)

            def emit_x_chunk(mc, kb, xg):
                tiles8, tilesb = xg
                xs = xstage.tile([128, M_CHUNK], F32, tag="xs")
                nc.sync.dma_start(
                    xs[:], xt_d[128 * kb:128 * (kb + 1),
                                M_CHUNK * mc:M_CHUNK * (mc + 1)])
                if kb < KF:
                    j = kb // 2
                    if kb % 2 == 0:
                        t8 = xwin.tile([128, 2, M_CHUNK], FP8,
                                       tag=f"x8_{j}", name=f"x8_{j}_{mc}")
                        tiles8[j] = t8
                    dst = tiles8[kb // 2][:, kb % 2, :]
                else:
                    tb = xwin.tile([128, M_CHUNK], BF16,
                                   tag=f"xb_{kb}", name=f"xb_{kb}_{mc}")
                    tilesb[kb] = tb
                    dst = tb[:]
                if kb % 2 == 0:
                    nc.scalar.mul(dst, xs[:], sh_col[:])
                else:
                    nc.vector.tensor_scalar(
                        dst, xs[:], sh_col[:], None, mybir.AluOpType.mult)

            def load_x_group(mc, inject=None):
                xg = ({}, {})
                for kb in range(KO):
                    if inject and kb in inject:
                        quant_half(*inject[kb])
                    emit_x_chunk(mc, kb, xg)
                return xg

            def chain(nb, mc, xg):
                tiles8, tilesb = xg
                ps = psum_pool.tile([128, M_CHUNK], F32, tag="ps",
                                    name=f"ps{nb}_{mc}")
                for j in range(N_PAIR):
                    nc.tensor.matmul(
                        ps[:], lhsT=qt8[nb][:, 2 * j:2 * j + 2, :],
                        rhs=tiles8[j][:],
                        start=(j == 0), stop=False,
                        perf_mode=mybir.MatmulPerfMode.DoubleRow)
                for i, kb in enumerate(range(KF, KO)):
                    nc.tensor.matmul(
                        ps[:], lhsT=qtb[nb][:, i, :], rhs=tilesb[kb][:],
                        start=(KF == 0 and i == 0), stop=(kb == KO - 1))
                ob = out_pool.tile([128, M_CHUNK], F32, tag="outp",
                                   name=f"ob{nb}_{mc}")
                nc.scalar.copy(ob[:], ps[:])
                nc.scalar.dma_start(
                    outT[128 * nb:128 * (nb + 1),
                         M_CHUNK * mc:M_CHUNK * (mc + 1)], ob[:])

            quant_half(0, 0)
            xg0 = load_x_group(0, inject={
                2: (0, 1), 5: (1, 0), 8: (1, 1), 12: (2, 0), 16: (2, 1),
                20: (3, 0), 24: (3, 1), 28: (4, 0)})
            xg1 = load_x_group(1, inject={
                0: (4, 1), 4: (5, 0), 8: (5, 1), 12: (6, 0), 16: (6, 1),
                20: (7, 0), 24: (7, 1), 28: (8, 0)})
            for nb in range(8):
                if nb < 7:
                    quant_half(nb + 8, 1)
                    quant_half(nb + 9, 0)
                chain(nb, 0, xg0)
                chain(nb, 1, xg1)
            quant_half(15, 1)
            for nb in range(8, N_NB):
                chain(nb, 0, xg0)
            xg2 = load_x_group(2)
            for nb in range(8, N_NB):
                chain(nb, 1, xg1)
            xg3 = load_x_group(3)
            for nb in range(N_NB):
                chain(nb, 2, xg2)
            for nb in range(N_NB):
                chain(nb, 3, xg3)

    nc.compile()
    return nc


def kernel(x, weight, bias):
    global LAST_RESULTS
    x = np.asarray(x, dtype=np.float32)
    weight = np.ascontiguousarray(np.asarray(weight, dtype=np.float32))
    bias = np.ascontiguousarray(np.asarray(bias, dtype=np.float32))

    if "nc_scale" not in _CACHE:
        _CACHE["nc_scale"] = _build_scale()
        _CACHE["nc_main"] = _build_main()
    nc_scale, nc_main = _CACHE["nc_scale"], _CACHE["nc_main"]

    trace = bool(int(os.environ.get("KERNEL_TRACE", "0")))
    kw = {"trace": True, "trace_cores": [0]} if trace else {}

    in_a = [{"wredN": weight[WRED * c:WRED * (c + 1)]}
            for c in range(N_CORES)]
    res_a = run_bass_kernel_spmd(nc_scale, in_a, list(range(N_CORES)), **kw)
    partials = np.array(
        [res_a.results[c]["partial"][0, 0] for c in range(N_CORES)],
        dtype=np.float32)

    xr = x.reshape(M_ALL, K)
    in_b = []
    for c in range(N_CORES):
        i, j = c // F_GRP, c % F_GRP
        w_sh = weight[N_SH * j:N_SH * (j + 1)]
        wt5 = np.ascontiguousarray(
            w_sh.reshape(N_NB, 128, KO, 128).transpose(0, 3, 2, 1))
        in_b.append({
            "xt_sh": np.ascontiguousarray(xr[M_SH * i:M_SH * (i + 1)].T),
            "wt5": wt5,
            "partials": partials,
        })
    res_b = run_bass_kernel_spmd(nc_main, in_b, list(range(N_CORES)), **kw)
    LAST_RESULTS = (res_a, res_b)

    out = np.empty((M_ALL, N_OUT), dtype=np.float32)
    for c in range(N_CORES):
        i, j = c // F_GRP, c % F_GRP
        out[M_SH * i:M_SH * (i + 1), N_SH * j:N_SH * (j + 1)] = \
            res_b.results[c]["outT"].T
    if bias.any():
        out += bias[None, :]
    return out.reshape(B, S, N_OUT)


# revision 17
# speedup vs baseline: 1.2730x; 1.1132x over previous
import os

import numpy as np

import concourse.bass as bass
import concourse.tile as tile
from concourse import bacc, mybir
from concourse.bass_utils import run_bass_kernel_spmd

N_CORES = 8
R_GRP, F_GRP = 4, 2
B, S, K = 4, 2048, 4096
N_OUT = 4096
M_ALL = B * S
M_SH = M_ALL // R_GRP
N_SH = N_OUT // F_GRP
WRED = N_OUT // N_CORES
KO = K // 128
M_CHUNK = 512
N_MC = M_SH // M_CHUNK
N_NB = N_SH // 128

N_PAIR = 8
KF = 2 * N_PAIR

C_ABS = float(np.float32(0.79788456))
THRESH = 2.0 / 3.0
F32 = mybir.dt.float32
BF16 = mybir.dt.bfloat16
FP8 = mybir.dt.float8e4

_CACHE = {}
LAST_RESULTS = None


def _build_scale():
    nc = bacc.Bacc(None, target_bir_lowering=False, num_devices=N_CORES)
    wred_d = nc.dram_tensor("wredN", [WRED, K], F32, kind="ExternalInput")
    part_d = nc.dram_tensor("partial", [1, 1], F32, kind="ExternalOutput")

    with tile.TileContext(nc) as tc:
        with (
            tc.tile_pool(name="misc", bufs=1) as misc,
            tc.tile_pool(name="redstage", bufs=4) as redstage,
            tc.tile_pool(name="psum_s", bufs=1, space="PSUM") as psum_s_pool,
        ):
            racc = misc.tile([128, 8], F32)
            for t in range(8):
                wf = redstage.tile([128, K // 2], F32, tag="redstage")
                (nc.sync if t % 2 == 0 else nc.scalar).dma_start(
                    wf[:], wred_d.rearrange("(a p) (b c) -> p a b c", p=128, b=2)
                    [:, t // 2, t % 2, :])
                if t % 2 == 0:
                    nc.vector.tensor_reduce(
                        racc[:, t:t + 1], wf[:],
                        axis=mybir.AxisListType.X, op=mybir.AluOpType.add,
                        apply_absolute_value=True)
                else:
                    nc.scalar.activation(
                        wf[:], wf[:], mybir.ActivationFunctionType.Abs,
                        accum_out=racc[:, t:t + 1])
            rsm = misc.tile([128, 8], F32)
            nc.vector.tensor_scalar(
                rsm[:], racc[:], -float(np.float32((K // 2) * np.float32(C_ABS))),
                None, mybir.AluOpType.add)
            r1 = misc.tile([128, 1], F32)
            nc.vector.tensor_reduce(
                r1[:], rsm[:], axis=mybir.AxisListType.X, op=mybir.AluOpType.add)
            ones_col = misc.tile([128, 1], F32)
            nc.vector.memset(ones_col[:], 1.0)
            ps1 = psum_s_pool.tile([1, 1], F32)
            nc.tensor.matmul(ps1[:], lhsT=r1[:], rhs=ones_col[:])
            sc = misc.tile([1, 1], F32)
            nc.vector.tensor_copy(sc[:], ps1[:])
            nc.sync.dma_start(part_d[:], sc[:])

    nc.compile()
    return nc


def _build_main():
    nc = bacc.Bacc(None, target_bir_lowering=False, num_devices=N_CORES)
    xt_d = nc.dram_tensor("xt_sh", [K, M_SH], F32, kind="ExternalInput")
    wt5_d = nc.dram_tensor("wt5", [N_NB, 128, KO, 128], F32, kind="ExternalInput")
    part_d = nc.dram_tensor("partials", [N_CORES], F32, kind="ExternalInput")
    outT = nc.dram_tensor("outT", [N_SH, M_SH], F32, kind="ExternalOutput")

    with tile.TileContext(nc) as tc:
        with (
            tc.tile_pool(name="misc", bufs=1) as misc,
            tc.tile_pool(name="xwin", bufs=2) as xwin,
            tc.tile_pool(name="xstage", bufs=3) as xstage,
            tc.tile_pool(name="wq", bufs=2) as wq_pool,
            tc.tile_pool(name="yq", bufs=2) as yq_pool,
            tc.tile_pool(name="qt", bufs=1) as qt_pool,
            tc.tile_pool(name="outp", bufs=3) as out_pool,
            tc.tile_pool(name="psum", bufs=7, space="PSUM") as psum_pool,
            tc.tile_pool(name="psum_s", bufs=1, space="PSUM") as psum_s_pool,
        ):
            pt = misc.tile([1, N_CORES], F32)
            nc.sync.dma_start(pt[:], part_d.rearrange("(p o) -> p o", p=1))
            s0 = misc.tile([1, 1], F32)
            nc.vector.tensor_reduce(
                s0[:], pt[:], axis=mybir.AxisListType.X, op=mybir.AluOpType.add)
            ones_row = misc.tile([1, 128], F32)
            nc.vector.memset(ones_row[:], 1.0)
            ps_bc = psum_s_pool.tile([128, 1], F32)
            nc.tensor.matmul(ps_bc[:], lhsT=ones_row[:], rhs=s0[:])
            mean_col = misc.tile([128, 1], F32)
            nc.vector.tensor_scalar(
                mean_col[:], ps_bc[:], 1.0 / (N_OUT * K), C_ABS,
                mybir.AluOpType.mult, mybir.AluOpType.add)
            s_col = misc.tile([128, 1], F32)
            nc.vector.tensor_scalar(
                s_col[:], mean_col[:], 1e-5, 1000.0,
                mybir.AluOpType.max, mybir.AluOpType.min)
            thr_col = misc.tile([128, 1], F32)
            nc.vector.tensor_scalar(
                thr_col[:], s_col[:], THRESH, None, mybir.AluOpType.mult)
            nthr_col = misc.tile([128, 1], F32)
            nc.vector.tensor_scalar(
                nthr_col[:], s_col[:], -THRESH, None, mybir.AluOpType.mult)
            sh_col = misc.tile([128, 1], F32)
            nc.vector.tensor_scalar(
                sh_col[:], s_col[:], 0.5, None, mybir.AluOpType.mult)

            qt8 = [qt_pool.tile([128, KF, 128], FP8, name=f"qt8_{nb}")
                   for nb in range(N_NB)] if KF else None
            qtb = [qt_pool.tile([128, KO - KF, 128], BF16, name=f"qtb_{nb}")
                   for nb in range(N_NB)]

            def quant_half(nb, h):
                wq = wq_pool.tile([128, KO // 2, 128], F32, tag="wq",
                                  name=f"wq{nb}_{h}")
                nc.sync.dma_start(wq[:], wt5_d[nb, :, 16 * h:16 * (h + 1), :])
                wq_f = wq[:].rearrange("p a b -> p (a b)")
                if h == 0:
                    sp = yq_pool.tile([128, 2048], BF16, tag="yq",
                                      name=f"sp{nb}")
                    nc.scalar.activation(
                        sp[:], wq_f, mybir.ActivationFunctionType.Sign,
                        bias=nthr_col[:])
                    sn = yq_pool.tile([128, 2048], BF16, tag="yq",
                                      name=f"sn{nb}")
                    nc.scalar.activation(
                        sn[:], wq_f, mybir.ActivationFunctionType.Sign,
                        bias=thr_col[:])
                    if KF:
                        nc.vector.tensor_tensor(
                            qt8[nb][:].rearrange("p a b -> p (a b)"),
                            sp[:, :KF * 128], sn[:, :KF * 128],
                            mybir.AluOpType.add)
                    if KF < 16:
                        nc.vector.tensor_tensor(
                            qtb[nb][:, :16 - KF, :].rearrange("p a b -> p (a b)"),
                            sp[:, KF * 128:], sn[:, KF * 128:],
                            mybir.AluOpType.add)
                else:
                    mpos = yq_pool.tile([128, 2048], BF16, tag="yq",
                                        name=f"mp{nb}")
                    nc.vector.tensor_scalar(
                        mpos[:], wq_f, thr_col[:], 2.0,
                        mybir.AluOpType.is_gt, mybir.AluOpType.mult)
                    mneg = yq_pool.tile([128, 2048], BF16, tag="yq",
                                        name=f"mn{nb}")
                    nc.vector.tensor_scalar(
                        mneg[:], wq_f, nthr_col[:], 2.0,
                        mybir.AluOpType.is_lt, mybir.AluOpType.mult)
                    nc.vector.tensor_tensor(
                        qtb[nb][:, 16 - KF:, :].rearrange("p a b -> p (a b)"),
                        mpos[:], mneg[:], mybir.AluOpType.subtract)

            def emit_x_chunk(mc, kb, xg):
                tiles8, tilesb = xg
                xs = xstage.tile([128, M_CHUNK], F32, tag="xs")
                nc.sync.dma_start(
                    xs[:], xt_d[128 * kb:128 * (kb + 1),
                                M_CHUNK * mc:M_CHUNK * (mc + 1)])
                if kb < KF:
                    j = kb // 2
                    if kb % 2 == 0:
                        t8 = xwin.tile([128, 2, M_CHUNK], FP8,
                                       tag=f"x8_{j}", name=f"x8_{j}_{mc}")
                        tiles8[j] = t8
                    dst = tiles8[kb // 2][:, kb % 2, :]
                else:
                    tb = xwin.tile([128, M_CHUNK], BF16,
                                   tag=f"xb_{kb}", name=f"xb_{kb}_{mc}")
                    tilesb[kb] = tb
                    dst = tb[:]
                if kb % 2 == 0:
                    nc.scalar.mul(dst, xs[:], sh_col[:])
                else:
                    nc.vector.tensor_scalar(
                        dst, xs[:], sh_col[:], None, mybir.AluOpType.mult)

            def load_x_group(mc, inject=None):
                xg = ({}, {})
                for kb in range(KO):
                    if inject and kb in inject:
                        quant_half(*inject[kb])
                    emit_x_chunk(mc, kb, xg)
                return xg

            def chain(nb, mc, xg):
                tiles8, tilesb = xg
                ps = psum_pool.tile([128, M_CHUNK], F32, tag="ps",
                                    name=f"ps{nb}_{mc}")
                for j in range(N_PAIR):
                    nc.tensor.matmul(
                        ps[:], lhsT=qt8[nb][:, 2 * j:2 * j + 2, :],
                        rhs=tiles8[j][:],
                        start=(j == 0), stop=False,
                        perf_mode=mybir.MatmulPerfMode.DoubleRow)
                for i, kb in enumerate(range(KF, KO)):
                    nc.tensor.matmul(
                        ps[:], lhsT=qtb[nb][:, i, :], rhs=tilesb[kb][:],
                        start=(KF == 0 and i == 0), stop=(kb == KO - 1))
                ob = out_pool.tile([128, M_CHUNK], F32, tag="outp",
                                   name=f"ob{nb}_{mc}")
                nc.scalar.copy(ob[:], ps[:])
                nc.scalar.dma_start(
                    outT[128 * nb:128 * (nb + 1),
                         M_CHUNK * mc:M_CHUNK * (mc + 1)], ob[:])

            quant_half(0, 0)
            quant_half(0, 1)
            xg0 = load_x_group(0, inject={
                4: (1, 0), 10: (1, 1), 16: (2, 0), 22: (2, 1), 28: (3, 0)})
            h_list = [(3, 1)] + [(q, h) for q in range(4, N_NB)
                                 for h in (0, 1)]
            hi = 0
            xg1 = ({}, {})
            xg2 = ({}, {})
            xg3 = ({}, {})

            def pace_quant(n):
                nonlocal_hi = n
                return nonlocal_hi

            for i in range(8):
                if hi < len(h_list):
                    quant_half(*h_list[hi]); hi += 1
                for c in range(4):
                    emit_x_chunk(1, 4 * i + c, xg1)
                chain(i, 0, xg0)
            for i in range(8, 16):
                for _ in range(3):
                    if hi < len(h_list):
                        quant_half(*h_list[hi]); hi += 1
                chain(i, 0, xg0)
                chain(i - 8, 1, xg1)
            for i in range(16, 24):
                for c in range(4):
                    emit_x_chunk(2, 4 * (i - 16) + c, xg2)
                chain(i - 8, 1, xg1)
            for nb in range(N_NB):
                if nb < 8:
                    for c in range(4):
                        emit_x_chunk(3, 4 * nb + c, xg3)
                chain(nb, 2, xg2)
            for nb in range(N_NB):
                chain(nb, 3, xg3)

    nc.compile()
    return nc


def _build_main():
    nc = bacc.Bacc(None, target_bir_lowering=False, num_devices=N_CORES)
    xt_d = nc.dram_tensor("xt_sh", [K, M_SH], F32, kind="ExternalInput")
    wt5_d = nc.dram_tensor("wt5", [N_NB, 128, KO, 128], F32, kind="ExternalInput")
    part_d = nc.dram_tensor("partials", [N_CORES], F32, kind="ExternalInput")
    outT = nc.dram_tensor("outT", [N_SH, M_SH], F32, kind="ExternalOutput")

    with tile.TileContext(nc) as tc:
        with (
            tc.tile_pool(name="misc", bufs=1) as misc,
            tc.tile_pool(name="xwin", bufs=2) as xwin,
            tc.tile_pool(name="xstage", bufs=3) as xstage,
            tc.tile_pool(name="wq", bufs=2) as wq_pool,
            tc.tile_pool(name="yq", bufs=2) as yq_pool,
            tc.tile_pool(name="qt", bufs=1) as qt_pool,
            tc.tile_pool(name="outp", bufs=3) as out_pool,
            tc.tile_pool(name="psum", bufs=7, space="PSUM") as psum_pool,
            tc.tile_pool(name="psum_s", bufs=1, space="PSUM") as psum_s_pool,
        ):
            pt = misc.tile([1, N_CORES], F32)
            nc.sync.dma_start(pt[:], part_d.rearrange("(p o) -> p o", p=1))
            s0 = misc.tile([1, 1], F32)
            nc.vector.tensor_reduce(
                s0[:], pt[:], axis=mybir.AxisListType.X, op=mybir.AluOpType.add)
            ones_row = misc.tile([1, 128], F32)
            nc.vector.memset(ones_row[:], 1.0)
            ps_bc = psum_s_pool.tile([128, 1], F32)
            nc.tensor.matmul(ps_bc[:], lhsT=ones_row[:], rhs=s0[:])
            mean_col = misc.tile([128, 1], F32)
            nc.vector.tensor_scalar(
                mean_col[:], ps_bc[:], 1.0 / (N_OUT * K), C_ABS,
                mybir.AluOpType.mult, mybir.AluOpType.add)
            s_col = misc.tile([128, 1], F32)
            nc.vector.tensor_scalar(
                s_col[:], mean_col[:], 1e-5, 1000.0,
                mybir.AluOpType.max, mybir.AluOpType.min)
            thr_col = misc.tile([128, 1], F32)
            nc.vector.tensor_scalar(
                thr_col[:], s_col[:], THRESH, None, mybir.AluOpType.mult)
            nthr_col = misc.tile([128, 1], F32)
            nc.vector.tensor_scalar(
                nthr_col[:], s_col[:], -THRESH, None, mybir.AluOpType.mult)
            sh_col = misc.tile([128, 1], F32)
            nc.vector.tensor_scalar(
                sh_col[:], s_col[:], 0.5, None, mybir.AluOpType.mult)

            qt8 = [qt_pool.tile([128, KF, 128], FP8, name=f"qt8_{nb}")
                   for nb in range(N_NB)] if KF else None
            qtb = [qt_pool.tile([128, KO - KF, 128], BF16, name=f"qtb_{nb}")
                   for nb in range(N_NB)]

            def quant_half(nb, h):
                wq = wq_pool.tile([128, KO // 2, 128], F32, tag="wq",
                                  name=f"wq{nb}_{h}")
                nc.sync.dma_start(wq[:], wt5_d[nb, :, 16 * h:16 * (h + 1), :])
                wq_f = wq[:].rearrange("p a b -> p (a b)")
                if h == 0:
                    sp = yq_pool.tile([128, 2048], BF16, tag="yq",
                                      name=f"sp{nb}")
                    nc.scalar.activation(
                        sp[:], wq_f, mybir.ActivationFunctionType.Sign,
                        bias=nthr_col[:])
                    sn = yq_pool.tile([128, 2048], BF16, tag="yq",
                                      name=f"sn{nb}")
                    nc.scalar.activation(
                        sn[:], wq_f, mybir.ActivationFunctionType.Sign,
                        bias=thr_col[:])
                    if KF:
                        nc.vector.tensor_tensor(
                            qt8[nb][:].rearrange("p a b -> p (a b)"),
                            sp[:, :KF * 128], sn[:, :KF * 128],
                            mybir.AluOpType.add)
                    if KF < 16:
                        nc.vector.tensor_tensor(
                            qtb[nb][:, :16 - KF, :].rearrange("p a b -> p (a b)"),
                            sp[:, KF * 128:], sn[:, KF * 128:],
                            mybir.AluOpType.add)
                else:
                    mpos = yq_pool.tile([128, 2048], BF16, tag="yq",
                                        name=f"mp{nb}")
                    nc.vector.tensor_scalar(
                        mpos[:], wq_f, thr_col[:], 2.0,
                        mybir.AluOpType.is_gt, mybir.AluOpType.mult)
                    mneg = yq_pool.tile([128, 2048], BF16, tag="yq",
                                        name=f"mn{nb}")
                    nc.vector.tensor_scalar(
                        mneg[:], wq_f, nthr_col[:], 2.0,
                        mybir.AluOpType.is_lt, mybir.AluOpType.mult)
                    nc.vector.tensor_tensor(
                        qtb[nb][:, 16 - KF:, :].rearrange("p a b -> p (a b)"),
                        mpos[:], mneg[:], mybir.AluOpType.subtract)

            def emit_x_chunk(mc, kb, xg):
                tiles8, tilesb = xg
                xs = xstage.tile([128, M_CHUNK], F32, tag="xs")
                nc.sync.dma_start(
                    xs[:], xt_d[128 * kb:128 * (kb + 1),
                                M_CHUNK * mc:M_CHUNK * (mc + 1)])
                if kb < KF:
                    j = kb // 2
                    if kb % 2 == 0:
                        t8 = xwin.tile([128, 2, M_CHUNK], FP8,
                                       tag=f"x8_{j}", name=f"x8_{j}_{mc}")
                        tiles8[j] = t8
                    dst = tiles8[kb // 2][:, kb % 2, :]
                else:
                    tb = xwin.tile([128, M_CHUNK], BF16,
                                   tag=f"xb_{kb}", name=f"xb_{kb}_{mc}")
                    tilesb[kb] = tb
                    dst = tb[:]
                if kb % 2 == 0:
                    nc.scalar.mul(dst, xs[:], sh_col[:])
                else:
                    nc.vector.tensor_scalar(
                        dst, xs[:], sh_col[:], None, mybir.AluOpType.mult)

            def load_x_group(mc, inject=None):
                xg = ({}, {})
                for kb in range(KO):
                    if inject and kb in inject:
                        quant_half(*inject[kb])
                    emit_x_chunk(mc, kb, xg)
                return xg

            def chain(nb, mc, xg):
                tiles8, tilesb = xg
                ps = psum_pool.tile([128, M_CHUNK], F32, tag="ps",
                                    name=f"ps{nb}_{mc}")
                for j in range(N_PAIR):
                    nc.tensor.matmul(
                        ps[:], lhsT=qt8[nb][:, 2 * j:2 * j + 2, :],
                        rhs=tiles8[j][:],
                        start=(j == 0), stop=False,
                        perf_mode=mybir.MatmulPerfMode.DoubleRow)
                for i, kb in enumerate(range(KF, KO)):
                    nc.tensor.matmul(
                        ps[:], lhsT=qtb[nb][:, i, :], rhs=tilesb[kb][:],
                        start=(KF == 0 and i == 0), stop=(kb == KO - 1))
                ob = out_pool.tile([128, M_CHUNK], F32, tag="outp",
                                   name=f"ob{nb}_{mc}")
                nc.scalar.copy(ob[:], ps[:])
                nc.scalar.dma_start(
                    outT[128 * nb:128 * (nb + 1),
                         M_CHUNK * mc:M_CHUNK * (mc + 1)], ob[:])

            quant_half(0, 0)
            quant_half(0, 1)
            xg0 = load_x_group(0, inject={
                4: (1, 0), 10: (1, 1), 16: (2, 0), 22: (2, 1), 28: (3, 0)})
            h_list = [(3, 1)] + [(q, h) for q in range(4, N_NB)
                                 for h in (0, 1)]
            hi = 0
            xg1 = ({}, {})
            for nb in range(N_NB):
                for _ in range(2):
                    if hi < len(h_list):
                        quant_half(*h_list[hi])
                        hi += 1
                emit_x_chunk(1, 2 * nb, xg1)
                emit_x_chunk(1, 2 * nb + 1, xg1)
                chain(nb, 0, xg0)
            xg2 = ({}, {})
            for nb in range(N_NB):
                emit_x_chunk(2, 2 * nb, xg2)
                emit_x_chunk(2, 2 * nb + 1, xg2)
                chain(nb, 1, xg1)
            xg3 = ({}, {})
            for nb in range(N_NB):
                emit_x_chunk(3, 2 * nb, xg3)
                emit_x_chunk(3, 2 * nb + 1, xg3)
                chain(nb, 2, xg2)
            for nb in range(N_NB):
                chain(nb, 3, xg3)

    nc.compile()
    return nc


def _build_main():
    nc = bacc.Bacc(None, target_bir_lowering=False, num_devices=N_CORES)
    xt_d = nc.dram_tensor("xt_sh", [K, M_SH], F32, kind="ExternalInput")
    wt5_d = nc.dram_tensor("wt5", [N_NB, 128, KO, 128], F32, kind="ExternalInput")
    part_d = nc.dram_tensor("partials", [N_CORES], F32, kind="ExternalInput")
    outT = nc.dram_tensor("outT", [N_SH, M_SH], F32, kind="ExternalOutput")

    with tile.TileContext(nc) as tc:
        with (
            tc.tile_pool(name="misc", bufs=1) as misc,
            tc.tile_pool(name="xwin", bufs=2) as xwin,
            tc.tile_pool(name="xstage", bufs=3) as xstage,
            tc.tile_pool(name="wq", bufs=2) as wq_pool,
            tc.tile_pool(name="yq", bufs=2) as yq_pool,
            tc.tile_pool(name="qt", bufs=1) as qt_pool,
            tc.tile_pool(name="outp", bufs=3) as out_pool,
            tc.tile_pool(name="psum", bufs=7, space="PSUM") as psum_pool,
            tc.tile_pool(name="psum_s", bufs=1, space="PSUM") as psum_s_pool,
        ):
            pt = misc.tile([1, N_CORES], F32)
            nc.sync.dma_start(pt[:], part_d.rearrange("(p o) -> p o", p=1))
            s0 = misc.tile([1, 1], F32)
            nc.vector.tensor_reduce(
                s0[:], pt[:], axis=mybir.AxisListType.X, op=mybir.AluOpType.add)
            ones_row = misc.tile([1, 128], F32)
            nc.vector.memset(ones_row[:], 1.0)
            ps_bc = psum_s_pool.tile([128, 1], F32)
            nc.tensor.matmul(ps_bc[:], lhsT=ones_row[:], rhs=s0[:])
            mean_col = misc.tile([128, 1], F32)
            nc.vector.tensor_scalar(
                mean_col[:], ps_bc[:], 1.0 / (N_OUT * K), C_ABS,
                mybir.AluOpType.mult, mybir.AluOpType.add)
            s_col = misc.tile([128, 1], F32)
            nc.vector.tensor_scalar(
                s_col[:], mean_col[:], 1e-5, 1000.0,
                mybir.AluOpType.max, mybir.AluOpType.min)
            thr_col = misc.tile([128, 1], F32)
            nc.vector.tensor_scalar(
                thr_col[:], s_col[:], THRESH, None, mybir.AluOpType.mult)
            nthr_col = misc.tile([128, 1], F32)
            nc.vector.tensor_scalar(
                nthr_col[:], s_col[:], -THRESH, None, mybir.AluOpType.mult)
            sh_col = misc.tile([128, 1], F32)
            nc.vector.tensor_scalar(
                sh_col[:], s_col[:], 0.5, None, mybir.AluOpType.mult)

            qt8 = [qt_pool.tile([128, KF, 128], FP8, name=f"qt8_{nb}")
                   for nb in range(N_NB)] if KF else None
            qtb = [qt_pool.tile([128, KO - KF, 128], BF16, name=f"qtb_{nb}")
                   for nb in range(N_NB)]

            def quant_half(nb, h):
                wq = wq_pool.tile([128, KO // 2, 128], F32, tag="wq",
                                  name=f"wq{nb}_{h}")
                nc.sync.dma_start(wq[:], wt5_d[nb, :, 16 * h:16 * (h + 1), :])
                wq_f = wq[:].rearrange("p a b -> p (a b)")
                if h == 0:
                    sp = yq_pool.tile([128, 2048], BF16, tag="yq",
                                      name=f"sp{nb}")
                    nc.scalar.activation(
                        sp[:], wq_f, mybir.ActivationFunctionType.Sign,
                        bias=nthr_col[:])
                    sn = yq_pool.tile([128, 2048], BF16, tag="yq",
                                      name=f"sn{nb}")
                    nc.scalar.activation(
                        sn[:], wq_f, mybir.ActivationFunctionType.Sign,
                        bias=thr_col[:])
                    if KF:
                        nc.vector.tensor_tensor(
                            qt8[nb][:].rearrange("p a b -> p (a b)"),
                            sp[:, :KF * 128], sn[:, :KF * 128],
                            mybir.AluOpType.add)
                    if KF < 16:
                        nc.vector.tensor_tensor(
                            qtb[nb][:, :16 - KF, :].rearrange("p a b -> p (a b)"),
                            sp[:, KF * 128:], sn[:, KF * 128:],
                            mybir.AluOpType.add)
                else:
                    mpos = yq_pool.tile([128, 2048], BF16, tag="yq",
                                        name=f"mp{nb}")
                    nc.vector.tensor_scalar(
                        mpos[:], wq_f, thr_col[:], 2.0,
                        mybir.AluOpType.is_gt, mybir.AluOpType.mult)
                    mneg = yq_pool.tile([128, 2048], BF16, tag="yq",
                                        name=f"mn{nb}")
                    nc.vector.tensor_scalar(
                        mneg[:], wq_f, nthr_col[:], 2.0,
                        mybir.AluOpType.is_lt, mybir.AluOpType.mult)
                    nc.vector.tensor_tensor(
                        qtb[nb][:, 16 - KF:, :].rearrange("p a b -> p (a b)"),
                        mpos[:], mneg[:], mybir.AluOpType.subtract)

            def emit_x_chunk(mc, kb, xg):
                tiles8, tilesb = xg
                xs = xstage.tile([128, M_CHUNK], F32, tag="xs")
                nc.sync.dma_start(
                    xs[:], xt_d[128 * kb:128 * (kb + 1),
                                M_CHUNK * mc:M_CHUNK * (mc + 1)])
                if kb < KF:
                    j = kb // 2
                    if kb % 2 == 0:
                        t8 = xwin.tile([128, 2, M_CHUNK], FP8,
                                       tag=f"x8_{j}", name=f"x8_{j}_{mc}")
                        tiles8[j] = t8
                    dst = tiles8[kb // 2][:, kb % 2, :]
                else:
                    tb = xwin.tile([128, M_CHUNK], BF16,
                                   tag=f"xb_{kb}", name=f"xb_{kb}_{mc}")
                    tilesb[kb] = tb
                    dst = tb[:]
                if kb % 2 == 0:
                    nc.scalar.mul(dst, xs[:], sh_col[:])
                else:
                    nc.vector.tensor_scalar(
                        dst, xs[:], sh_col[:], None, mybir.AluOpType.mult)

            def load_x_group(mc, inject=None):
                xg = ({}, {})
                for kb in range(KO):
                    if inject and kb in inject:
                        quant_half(*inject[kb])
                    emit_x_chunk(mc, kb, xg)
                return xg

            def chain(nb, mc, xg):
                tiles8, tilesb = xg
                ps = psum_pool.tile([128, M_CHUNK], F32, tag="ps",
                                    name=f"ps{nb}_{mc}")
                for j in range(N_PAIR):
                    nc.tensor.matmul(
                        ps[:], lhsT=qt8[nb][:, 2 * j:2 * j + 2, :],
                        rhs=tiles8[j][:],
                        start=(j == 0), stop=False,
                        perf_mode=mybir.MatmulPerfMode.DoubleRow)
                for i, kb in enumerate(range(KF, KO)):
                    nc.tensor.matmul(
                        ps[:], lhsT=qtb[nb][:, i, :], rhs=tilesb[kb][:],
                        start=(KF == 0 and i == 0), stop=(kb == KO - 1))
                ob = out_pool.tile([128, M_CHUNK], F32, tag="outp",
                                   name=f"ob{nb}_{mc}")
                nc.scalar.copy(ob[:], ps[:])
                nc.scalar.dma_start(
                    outT[128 * nb:128 * (nb + 1),
                         M_CHUNK * mc:M_CHUNK * (mc + 1)], ob[:])

            quant_half(0, 0)
            xg0 = load_x_group(0, inject={
                2: (0, 1), 5: (1, 0), 8: (1, 1), 12: (2, 0), 16: (2, 1),
                20: (3, 0), 24: (3, 1), 28: (4, 0)})
            xg1 = load_x_group(1, inject={
                0: (4, 1), 4: (5, 0), 8: (5, 1), 12: (6, 0), 16: (6, 1),
                20: (7, 0), 24: (7, 1), 28: (8, 0)})
            for nb in range(8):
                if nb < 7:
                    quant_half(nb + 8, 1)
                    quant_half(nb + 9, 0)
                chain(nb, 0, xg0)
                chain(nb, 1, xg1)
            quant_half(15, 1)
            for nb in range(8, N_NB):
                chain(nb, 0, xg0)
            xg2 = load_x_group(2)
            for nb in range(8, N_NB):
                chain(nb, 1, xg1)
            xg3 = load_x_group(3)
            for nb in range(N_NB):
                chain(nb, 2, xg2)
            for nb in range(N_NB):
                chain(nb, 3, xg3)

    nc.compile()
    return nc


def kernel(x, weight, bias):
    global LAST_RESULTS
    x = np.asarray(x, dtype=np.float32)
    weight = np.ascontiguousarray(np.asarray(weight, dtype=np.float32))
    bias = np.ascontiguousarray(np.asarray(bias, dtype=np.float32))

    if "nc_scale" not in _CACHE:
        _CACHE["nc_scale"] = _build_scale()
        _CACHE["nc_main"] = _build_main()
    nc_scale, nc_main = _CACHE["nc_scale"], _CACHE["nc_main"]

    trace = bool(int(os.environ.get("KERNEL_TRACE", "0")))
    kw = {"trace": True, "trace_cores": [0]} if trace else {}

    in_a = [{"wredN": weight[WRED * c:WRED * (c + 1)]}
            for c in range(N_CORES)]
    res_a = run_bass_kernel_spmd(nc_scale, in_a, list(range(N_CORES)), **kw)
    partials = np.array(
        [res_a.results[c]["partial"][0, 0] for c in range(N_CORES)],
        dtype=np.float32)

    xr = x.reshape(M_ALL, K)
    in_b = []
    for c in range(N_CORES):
        i, j = c // F_GRP, c % F_GRP
        w_sh = weight[N_SH * j:N_SH * (j + 1)]
        wt5 = np.ascontiguousarray(
            w_sh.reshape(N_NB, 128, KO, 128).transpose(0, 3, 2, 1))
        in_b.append({
            "xt_sh": np.ascontiguousarray(xr[M_SH * i:M_SH * (i + 1)].T),
            "wt5": wt5,
            "partials": partials,
        })
    res_b = run_bass_kernel_spmd(nc_main, in_b, list(range(N_CORES)), **kw)
    LAST_RESULTS = (res_a, res_b)

    out = np.empty((M_ALL, N_OUT), dtype=np.float32)
    for c in range(N_CORES):
        i, j = c // F_GRP, c % F_GRP
        out[M_SH * i:M_SH * (i + 1), N_SH * j:N_SH * (j + 1)] = \
            res_b.results[c]["outT"].T
    if bias.any():
        out += bias[None, :]
    return out.reshape(B, S, N_OUT)


# revision 18
# speedup vs baseline: 1.3405x; 1.0530x over previous
import os

import numpy as np

import concourse.bass as bass
import concourse.tile as tile
from concourse import bacc, mybir
from concourse.bass_utils import run_bass_kernel_spmd

N_CORES = 8
R_GRP, F_GRP = 4, 2
B, S, K = 4, 2048, 4096
N_OUT = 4096
M_ALL = B * S
M_SH = M_ALL // R_GRP
N_SH = N_OUT // F_GRP
WRED = N_OUT // N_CORES
KO = K // 128
M_CHUNK = 512
N_MC = M_SH // M_CHUNK
N_NB = N_SH // 128

N_PAIR = 8
KF = 2 * N_PAIR

C_ABS = float(np.float32(0.79788456))
THRESH = 2.0 / 3.0
F32 = mybir.dt.float32
BF16 = mybir.dt.bfloat16
FP8 = mybir.dt.float8e4

_CACHE = {}
LAST_RESULTS = None


def _build_scale():
    nc = bacc.Bacc(None, target_bir_lowering=False, num_devices=N_CORES)
    wred_d = nc.dram_tensor("wredN", [WRED, K], F32, kind="ExternalInput")
    part_d = nc.dram_tensor("partial", [1, 1], F32, kind="ExternalOutput")

    with tile.TileContext(nc) as tc:
        with (
            tc.tile_pool(name="misc", bufs=1) as misc,
            tc.tile_pool(name="redstage", bufs=4) as redstage,
            tc.tile_pool(name="psum_s", bufs=1, space="PSUM") as psum_s_pool,
        ):
            racc = misc.tile([128, 8], F32)
            for t in range(8):
                wf = redstage.tile([128, K // 2], F32, tag="redstage")
                (nc.sync if t % 2 == 0 else nc.scalar).dma_start(
                    wf[:], wred_d.rearrange("(a p) (b c) -> p a b c", p=128, b=2)
                    [:, t // 2, t % 2, :])
                if t % 2 == 0:
                    nc.vector.tensor_reduce(
                        racc[:, t:t + 1], wf[:],
                        axis=mybir.AxisListType.X, op=mybir.AluOpType.add,
                        apply_absolute_value=True)
                else:
                    nc.scalar.activation(
                        wf[:], wf[:], mybir.ActivationFunctionType.Abs,
                        accum_out=racc[:, t:t + 1])
            rsm = misc.tile([128, 8], F32)
            nc.vector.tensor_scalar(
                rsm[:], racc[:], -float(np.float32((K // 2) * np.float32(C_ABS))),
                None, mybir.AluOpType.add)
            r1 = misc.tile([128, 1], F32)
            nc.vector.tensor_reduce(
                r1[:], rsm[:], axis=mybir.AxisListType.X, op=mybir.AluOpType.add)
            ones_col = misc.tile([128, 1], F32)
            nc.vector.memset(ones_col[:], 1.0)
            ps1 = psum_s_pool.tile([1, 1], F32)
            nc.tensor.matmul(ps1[:], lhsT=r1[:], rhs=ones_col[:])
            sc = misc.tile([1, 1], F32)
            nc.vector.tensor_copy(sc[:], ps1[:])
            nc.sync.dma_start(part_d[:], sc[:])

    nc.compile()
    return nc


def _build_main():
    nc = bacc.Bacc(None, target_bir_lowering=False, num_devices=N_CORES)
    xt_d = nc.dram_tensor("xt_sh", [K, M_SH], F32, kind="ExternalInput")
    wt5_d = nc.dram_tensor("wt5", [N_NB, 128, KO, 128], F32, kind="ExternalInput")
    part_d = nc.dram_tensor("partials", [N_CORES], F32, kind="ExternalInput")
    outT = nc.dram_tensor("outT", [N_SH, M_SH], F32, kind="ExternalOutput")

    with tile.TileContext(nc) as tc:
        with (
            tc.tile_pool(name="misc", bufs=1) as misc,
            tc.tile_pool(name="xwin", bufs=2) as xwin,
            tc.tile_pool(name="xstage", bufs=4) as xstage,
            tc.tile_pool(name="wq", bufs=3) as wq_pool,
            tc.tile_pool(name="yq", bufs=3) as yq_pool,
            tc.tile_pool(name="qt", bufs=1) as qt_pool,
            tc.tile_pool(name="outp", bufs=3) as out_pool,
            tc.tile_pool(name="psum", bufs=7, space="PSUM") as psum_pool,
            tc.tile_pool(name="psum_s", bufs=1, space="PSUM") as psum_s_pool,
        ):
            pt = misc.tile([1, N_CORES], F32)
            nc.sync.dma_start(pt[:], part_d.rearrange("(p o) -> p o", p=1))
            s0 = misc.tile([1, 1], F32)
            nc.vector.tensor_reduce(
                s0[:], pt[:], axis=mybir.AxisListType.X, op=mybir.AluOpType.add)
            ones_row = misc.tile([1, 128], F32)
            nc.vector.memset(ones_row[:], 1.0)
            ps_bc = psum_s_pool.tile([128, 1], F32)
            nc.tensor.matmul(ps_bc[:], lhsT=ones_row[:], rhs=s0[:])
            mean_col = misc.tile([128, 1], F32)
            nc.vector.tensor_scalar(
                mean_col[:], ps_bc[:], 1.0 / (N_OUT * K), C_ABS,
                mybir.AluOpType.mult, mybir.AluOpType.add)
            s_col = misc.tile([128, 1], F32)
            nc.vector.tensor_scalar(
                s_col[:], mean_col[:], 1e-5, 1000.0,
                mybir.AluOpType.max, mybir.AluOpType.min)
            thr_col = misc.tile([128, 1], F32)
            nc.vector.tensor_scalar(
                thr_col[:], s_col[:], THRESH, None, mybir.AluOpType.mult)
            nthr_col = misc.tile([128, 1], F32)
            nc.vector.tensor_scalar(
                nthr_col[:], s_col[:], -THRESH, None, mybir.AluOpType.mult)
            sh_col = misc.tile([128, 1], F32)
            nc.vector.tensor_scalar(
                sh_col[:], s_col[:], 0.5, None, mybir.AluOpType.mult)

            qt8 = [qt_pool.tile([128, KF, 128], FP8, name=f"qt8_{nb}")
                   for nb in range(N_NB)] if KF else None
            qtb = [qt_pool.tile([128, KO - KF, 128], BF16, name=f"qtb_{nb}")
                   for nb in range(N_NB)]

            def quant_half(nb, h):
                wq = wq_pool.tile([128, KO // 2, 128], F32, tag="wq",
                                  name=f"wq{nb}_{h}")
                nc.sync.dma_start(wq[:], wt5_d[nb, :, 16 * h:16 * (h + 1), :])
                wq_f = wq[:].rearrange("p a b -> p (a b)")
                if h == 0:
                    sp = yq_pool.tile([128, 2048], BF16, tag="yq",
                                      name=f"sp{nb}")
                    nc.scalar.activation(
                        sp[:], wq_f, mybir.ActivationFunctionType.Sign,
                        bias=nthr_col[:])
                    sn = yq_pool.tile([128, 2048], BF16, tag="yq",
                                      name=f"sn{nb}")
                    nc.scalar.activation(
                        sn[:], wq_f, mybir.ActivationFunctionType.Sign,
                        bias=thr_col[:])
                    if KF:
                        nc.vector.tensor_tensor(
                            qt8[nb][:].rearrange("p a b -> p (a b)"),
                            sp[:, :KF * 128], sn[:, :KF * 128],
                            mybir.AluOpType.add)
                    if KF < 16:
                        nc.vector.tensor_tensor(
                            qtb[nb][:, :16 - KF, :].rearrange("p a b -> p (a b)"),
                            sp[:, KF * 128:], sn[:, KF * 128:],
                            mybir.AluOpType.add)
                else:
                    mpos = yq_pool.tile([128, 2048], BF16, tag="yq",
                                        name=f"mp{nb}")
                    nc.vector.tensor_scalar(
                        mpos[:], wq_f, thr_col[:], 2.0,
                        mybir.AluOpType.is_gt, mybir.AluOpType.mult)
                    mneg = yq_pool.tile([128, 2048], BF16, tag="yq",
                                        name=f"mn{nb}")
                    nc.vector.tensor_scalar(
                        mneg[:], wq_f, nthr_col[:], 2.0,
                        mybir.AluOpType.is_lt, mybir.AluOpType.mult)
                    nc.vector.tensor_tensor(
                        qtb[nb][:, 16 - KF:, :].rearrange("p a b -> p (a b)"),
                        mpos[:], mneg[:], mybir.AluOpType.subtract)

            def emit_x_chunk(mc, kb, xg):
                tiles8, tilesb = xg
                xs = xstage.tile([128, M_CHUNK], F32, tag="xs")
                nc.sync.dma_start(
                    xs[:], xt_d[128 * kb:128 * (kb + 1),
                                M_CHUNK * mc:M_CHUNK * (mc + 1)])
                if kb < KF:
                    j = kb // 2
                    if kb % 2 == 0:
                        t8 = xwin.tile([128, 2, M_CHUNK], FP8,
                                       tag=f"x8_{j}", name=f"x8_{j}_{mc}")
                        tiles8[j] = t8
                    dst = tiles8[kb // 2][:, kb % 2, :]
                else:
                    tb = xwin.tile([128, M_CHUNK], BF16,
                                   tag=f"xb_{kb}", name=f"xb_{kb}_{mc}")
                    tilesb[kb] = tb
                    dst = tb[:]
                if kb % 2 == 0:
                    nc.scalar.mul(dst, xs[:], sh_col[:])
                else:
                    nc.vector.tensor_scalar(
                        dst, xs[:], sh_col[:], None, mybir.AluOpType.mult)

            def load_x_group(mc, inject=None):
                xg = ({}, {})
                for kb in range(KO):
                    if inject and kb in inject:
                        quant_half(*inject[kb])
                    emit_x_chunk(mc, kb, xg)
                return xg

            def chain(nb, mc, xg):
                tiles8, tilesb = xg
                ps = psum_pool.tile([128, M_CHUNK], F32, tag="ps",
                                    name=f"ps{nb}_{mc}")
                for j in range(N_PAIR):
                    nc.tensor.matmul(
                        ps[:], lhsT=qt8[nb][:, 2 * j:2 * j + 2, :],
                        rhs=tiles8[j][:],
                        start=(j == 0), stop=False,
                        perf_mode=mybir.MatmulPerfMode.DoubleRow)
                for i, kb in enumerate(range(KF, KO)):
                    nc.tensor.matmul(
                        ps[:], lhsT=qtb[nb][:, i, :], rhs=tilesb[kb][:],
                        start=(KF == 0 and i == 0), stop=(kb == KO - 1))
                ob = out_pool.tile([128, M_CHUNK], F32, tag="outp",
                                   name=f"ob{nb}_{mc}")
                nc.scalar.copy(ob[:], ps[:])
                nc.scalar.dma_start(
                    outT[128 * nb:128 * (nb + 1),
                         M_CHUNK * mc:M_CHUNK * (mc + 1)], ob[:])

            quant_half(0, 0)
            quant_half(0, 1)
            xg0 = load_x_group(0, inject={
                4: (1, 0), 9: (1, 1), 14: (2, 0), 19: (2, 1), 24: (3, 0),
                28: (3, 1)})
            h_list = [(q, h) for q in range(4, N_NB) for h in (0, 1)]
            hi = 0
            xg1 = ({}, {})
            xg2 = ({}, {})
            xg3 = ({}, {})
            for i in range(8):
                for _ in range(2 if i % 2 == 0 else 1):
                    if hi < len(h_list):
                        quant_half(*h_list[hi]); hi += 1
                for c in range(4):
                    emit_x_chunk(1, 4 * i + c, xg1)
                chain(i, 0, xg0)
            for i in range(8, 16):
                for _ in range(2):
                    if hi < len(h_list):
                        quant_half(*h_list[hi]); hi += 1
                chain(i, 0, xg0)
                chain(i - 8, 1, xg1)
            for i in range(16, 24):
                for c in range(4):
                    emit_x_chunk(2, 4 * (i - 16) + c, xg2)
                chain(i - 8, 1, xg1)
            for nb in range(N_NB):
                if nb < 8:
                    for c in range(4):
                        emit_x_chunk(3, 4 * nb + c, xg3)
                chain(nb, 2, xg2)
            for nb in range(N_NB):
                chain(nb, 3, xg3)

    nc.compile()
    return nc


def _build_main():
    nc = bacc.Bacc(None, target_bir_lowering=False, num_devices=N_CORES)
    xt_d = nc.dram_tensor("xt_sh", [K, M_SH], F32, kind="ExternalInput")
    wt5_d = nc.dram_tensor("wt5", [N_NB, 128, KO, 128], F32, kind="ExternalInput")
    part_d = nc.dram_tensor("partials", [N_CORES], F32, kind="ExternalInput")
    outT = nc.dram_tensor("outT", [N_SH, M_SH], F32, kind="ExternalOutput")

    with tile.TileContext(nc) as tc:
        with (
            tc.tile_pool(name="misc", bufs=1) as misc,
            tc.tile_pool(name="xwin", bufs=2) as xwin,
            tc.tile_pool(name="xstage", bufs=4) as xstage,
            tc.tile_pool(name="wq", bufs=3) as wq_pool,
            tc.tile_pool(name="yq", bufs=3) as yq_pool,
            tc.tile_pool(name="qt", bufs=1) as qt_pool,
            tc.tile_pool(name="outp", bufs=3) as out_pool,
            tc.tile_pool(name="psum", bufs=7, space="PSUM") as psum_pool,
            tc.tile_pool(name="psum_s", bufs=1, space="PSUM") as psum_s_pool,
        ):
            pt = misc.tile([1, N_CORES], F32)
            nc.sync.dma_start(pt[:], part_d.rearrange("(p o) -> p o", p=1))
            s0 = misc.tile([1, 1], F32)
            nc.vector.tensor_reduce(
                s0[:], pt[:], axis=mybir.AxisListType.X, op=mybir.AluOpType.add)
            ones_row = misc.tile([1, 128], F32)
            nc.vector.memset(ones_row[:], 1.0)
            ps_bc = psum_s_pool.tile([128, 1], F32)
            nc.tensor.matmul(ps_bc[:], lhsT=ones_row[:], rhs=s0[:])
            mean_col = misc.tile([128, 1], F32)
            nc.vector.tensor_scalar(
                mean_col[:], ps_bc[:], 1.0 / (N_OUT * K), C_ABS,
                mybir.AluOpType.mult, mybir.AluOpType.add)
            s_col = misc.tile([128, 1], F32)
            nc.vector.tensor_scalar(
                s_col[:], mean_col[:], 1e-5, 1000.0,
                mybir.AluOpType.max, mybir.AluOpType.min)
            thr_col = misc.tile([128, 1], F32)
            nc.vector.tensor_scalar(
                thr_col[:], s_col[:], THRESH, None, mybir.AluOpType.mult)
            nthr_col = misc.tile([128, 1], F32)
            nc.vector.tensor_scalar(
                nthr_col[:], s_col[:], -THRESH, None, mybir.AluOpType.mult)
            sh_col = misc.tile([128, 1], F32)
            nc.vector.tensor_scalar(
                sh_col[:], s_col[:], 0.5, None, mybir.AluOpType.mult)

            qt8 = [qt_pool.tile([128, KF, 128], FP8, name=f"qt8_{nb}")
                   for nb in range(N_NB)] if KF else None
            qtb = [qt_pool.tile([128, KO - KF, 128], BF16, name=f"qtb_{nb}")
                   for nb in range(N_NB)]

            def quant_half(nb, h):
                wq = wq_pool.tile([128, KO // 2, 128], F32, tag="wq",
                                  name=f"wq{nb}_{h}")
                nc.sync.dma_start(wq[:], wt5_d[nb, :, 16 * h:16 * (h + 1), :])
                wq_f = wq[:].rearrange("p a b -> p (a b)")
                if h == 0:
                    sp = yq_pool.tile([128, 2048], BF16, tag="yq",
                                      name=f"sp{nb}")
                    nc.scalar.activation(
                        sp[:], wq_f, mybir.ActivationFunctionType.Sign,
                        bias=nthr_col[:])
                    sn = yq_pool.tile([128, 2048], BF16, tag="yq",
                                      name=f"sn{nb}")
                    nc.scalar.activation(
                        sn[:], wq_f, mybir.ActivationFunctionType.Sign,
                        bias=thr_col[:])
                    if KF:
                        nc.vector.tensor_tensor(
                            qt8[nb][:].rearrange("p a b -> p (a b)"),
                            sp[:, :KF * 128], sn[:, :KF * 128],
                            mybir.AluOpType.add)
                    if KF < 16:
                        nc.vector.tensor_tensor(
                            qtb[nb][:, :16 - KF, :].rearrange("p a b -> p (a b)"),
                            sp[:, KF * 128:], sn[:, KF * 128:],
                            mybir.AluOpType.add)
                else:
                    mpos = yq_pool.tile([128, 2048], BF16, tag="yq",
                                        name=f"mp{nb}")
                    nc.vector.tensor_scalar(
                        mpos[:], wq_f, thr_col[:], 2.0,
                        mybir.AluOpType.is_gt, mybir.AluOpType.mult)
                    mneg = yq_pool.tile([128, 2048], BF16, tag="yq",
                                        name=f"mn{nb}")
                    nc.vector.tensor_scalar(
                        mneg[:], wq_f, nthr_col[:], 2.0,
                        mybir.AluOpType.is_lt, mybir.AluOpType.mult)
                    nc.vector.tensor_tensor(
                        qtb[nb][:, 16 - KF:, :].rearrange("p a b -> p (a b)"),
                        mpos[:], mneg[:], mybir.AluOpType.subtract)

            def emit_x_chunk(mc, kb, xg):
                tiles8, tilesb = xg
                xs = xstage.tile([128, M_CHUNK], F32, tag="xs")
                nc.sync.dma_start(
                    xs[:], xt_d[128 * kb:128 * (kb + 1),
                                M_CHUNK * mc:M_CHUNK * (mc + 1)])
                if kb < KF:
                    j = kb // 2
                    if kb % 2 == 0:
                        t8 = xwin.tile([128, 2, M_CHUNK], FP8,
                                       tag=f"x8_{j}", name=f"x8_{j}_{mc}")
                        tiles8[j] = t8
                    dst = tiles8[kb // 2][:, kb % 2, :]
                else:
                    tb = xwin.tile([128, M_CHUNK], BF16,
                                   tag=f"xb_{kb}", name=f"xb_{kb}_{mc}")
                    tilesb[kb] = tb
                    dst = tb[:]
                if kb % 2 == 0:
                    nc.scalar.mul(dst, xs[:], sh_col[:])
                else:
                    nc.vector.tensor_scalar(
                        dst, xs[:], sh_col[:], None, mybir.AluOpType.mult)

            def load_x_group(mc, inject=None):
                xg = ({}, {})
                for kb in range(KO):
                    if inject and kb in inject:
                        quant_half(*inject[kb])
                    emit_x_chunk(mc, kb, xg)
                return xg

            def chain(nb, mc, xg):
                tiles8, tilesb = xg
                ps = psum_pool.tile([128, M_CHUNK], F32, tag="ps",
                                    name=f"ps{nb}_{mc}")
                for j in range(N_PAIR):
                    nc.tensor.matmul(
                        ps[:], lhsT=qt8[nb][:, 2 * j:2 * j + 2, :],
                        rhs=tiles8[j][:],
                        start=(j == 0), stop=False,
                        perf_mode=mybir.MatmulPerfMode.DoubleRow)
                for i, kb in enumerate(range(KF, KO)):
                    nc.tensor.matmul(
                        ps[:], lhsT=qtb[nb][:, i, :], rhs=tilesb[kb][:],
                        start=(KF == 0 and i == 0), stop=(kb == KO - 1))
                ob = out_pool.tile([128, M_CHUNK], F32, tag="outp",
                                   name=f"ob{nb}_{mc}")
                nc.scalar.copy(ob[:], ps[:])
                nc.scalar.dma_start(
                    outT[128 * nb:128 * (nb + 1),
                         M_CHUNK * mc:M_CHUNK * (mc + 1)], ob[:])

            quant_half(0, 0)
            quant_half(0, 1)
            xg0 = load_x_group(0, inject={
                4: (1, 0), 10: (1, 1), 16: (2, 0), 22: (2, 1), 28: (3, 0)})
            h_list = [(3, 1)] + [(q, h) for q in range(4, N_NB)
                                 for h in (0, 1)]
            hi = 0
            xg1 = ({}, {})
            for nb in range(N_NB):
                for _ in range(2):
                    if hi < len(h_list):
                        quant_half(*h_list[hi])
                        hi += 1
                emit_x_chunk(1, 2 * nb, xg1)
                emit_x_chunk(1, 2 * nb + 1, xg1)
                chain(nb, 0, xg0)
            xg2 = ({}, {})
            for nb in range(N_NB):
                emit_x_chunk(2, 2 * nb, xg2)
                emit_x_chunk(2, 2 * nb + 1, xg2)
                chain(nb, 1, xg1)
            xg3 = ({}, {})
            for nb in range(N_NB):
                emit_x_chunk(3, 2 * nb, xg3)
                emit_x_chunk(3, 2 * nb + 1, xg3)
                chain(nb, 2, xg2)
            for nb in range(N_NB):
                chain(nb, 3, xg3)

    nc.compile()
    return nc


def _build_main():
    nc = bacc.Bacc(None, target_bir_lowering=False, num_devices=N_CORES)
    xt_d = nc.dram_tensor("xt_sh", [K, M_SH], F32, kind="ExternalInput")
    wt5_d = nc.dram_tensor("wt5", [N_NB, 128, KO, 128], F32, kind="ExternalInput")
    part_d = nc.dram_tensor("partials", [N_CORES], F32, kind="ExternalInput")
    outT = nc.dram_tensor("outT", [N_SH, M_SH], F32, kind="ExternalOutput")

    with tile.TileContext(nc) as tc:
        with (
            tc.tile_pool(name="misc", bufs=1) as misc,
            tc.tile_pool(name="xwin", bufs=2) as xwin,
            tc.tile_pool(name="xstage", bufs=4) as xstage,
            tc.tile_pool(name="wq", bufs=3) as wq_pool,
            tc.tile_pool(name="yq", bufs=3) as yq_pool,
            tc.tile_pool(name="qt", bufs=1) as qt_pool,
            tc.tile_pool(name="outp", bufs=3) as out_pool,
            tc.tile_pool(name="psum", bufs=7, space="PSUM") as psum_pool,
            tc.tile_pool(name="psum_s", bufs=1, space="PSUM") as psum_s_pool,
        ):
            pt = misc.tile([1, N_CORES], F32)
            nc.sync.dma_start(pt[:], part_d.rearrange("(p o) -> p o", p=1))
            s0 = misc.tile([1, 1], F32)
            nc.vector.tensor_reduce(
                s0[:], pt[:], axis=mybir.AxisListType.X, op=mybir.AluOpType.add)
            ones_row = misc.tile([1, 128], F32)
            nc.vector.memset(ones_row[:], 1.0)
            ps_bc = psum_s_pool.tile([128, 1], F32)
            nc.tensor.matmul(ps_bc[:], lhsT=ones_row[:], rhs=s0[:])
            mean_col = misc.tile([128, 1], F32)
            nc.vector.tensor_scalar(
                mean_col[:], ps_bc[:], 1.0 / (N_OUT * K), C_ABS,
                mybir.AluOpType.mult, mybir.AluOpType.add)
            s_col = misc.tile([128, 1], F32)
            nc.vector.tensor_scalar(
                s_col[:], mean_col[:], 1e-5, 1000.0,
                mybir.AluOpType.max, mybir.AluOpType.min)
            thr_col = misc.tile([128, 1], F32)
            nc.vector.tensor_scalar(
                thr_col[:], s_col[:], THRESH, None, mybir.AluOpType.mult)
            nthr_col = misc.tile([128, 1], F32)
            nc.vector.tensor_scalar(
                nthr_col[:], s_col[:], -THRESH, None, mybir.AluOpType.mult)
            sh_col = misc.tile([128, 1], F32)
            nc.vector.tensor_scalar(
                sh_col[:], s_col[:], 0.5, None, mybir.AluOpType.mult)

            qt8 = [qt_pool.tile([128, KF, 128], FP8, name=f"qt8_{nb}")
                   for nb in range(N_NB)] if KF else None
            qtb = [qt_pool.tile([128, KO - KF, 128], BF16, name=f"qtb_{nb}")
                   for nb in range(N_NB)]

            def quant_half(nb, h):
                wq = wq_pool.tile([128, KO // 2, 128], F32, tag="wq",
                                  name=f"wq{nb}_{h}")
                nc.sync.dma_start(wq[:], wt5_d[nb, :, 16 * h:16 * (h + 1), :])
                wq_f = wq[:].rearrange("p a b -> p (a b)")
                if h == 0:
                    sp = yq_pool.tile([128, 2048], BF16, tag="yq",
                                      name=f"sp{nb}")
                    nc.scalar.activation(
                        sp[:], wq_f, mybir.ActivationFunctionType.Sign,
                        bias=nthr_col[:])
                    sn = yq_pool.tile([128, 2048], BF16, tag="yq",
                                      name=f"sn{nb}")
                    nc.scalar.activation(
                        sn[:], wq_f, mybir.ActivationFunctionType.Sign,
                        bias=thr_col[:])
                    if KF:
                        nc.vector.tensor_tensor(
                            qt8[nb][:].rearrange("p a b -> p (a b)"),
                            sp[:, :KF * 128], sn[:, :KF * 128],
                            mybir.AluOpType.add)
                    if KF < 16:
                        nc.vector.tensor_tensor(
                            qtb[nb][:, :16 - KF, :].rearrange("p a b -> p (a b)"),
                            sp[:, KF * 128:], sn[:, KF * 128:],
                            mybir.AluOpType.add)
                else:
                    mpos = yq_pool.tile([128, 2048], BF16, tag="yq",
                                        name=f"mp{nb}")
                    nc.vector.tensor_scalar(
                        mpos[:], wq_f, thr_col[:], 2.0,
                        mybir.AluOpType.is_gt, mybir.AluOpType.mult)
                    mneg = yq_pool.tile([128, 2048], BF16, tag="yq",
                                        name=f"mn{nb}")
                    nc.vector.tensor_scalar(
                        mneg[:], wq_f, nthr_col[:], 2.0,
                        mybir.AluOpType.is_lt, mybir.AluOpType.mult)
                    nc.vector.tensor_tensor(
                        qtb[nb][:, 16 - KF:, :].rearrange("p a b -> p (a b)"),
                        mpos[:], mneg[:], mybir.AluOpType.subtract)

            def emit_x_chunk(mc, kb, xg):
                tiles8, tilesb = xg
                xs = xstage.tile([128, M_CHUNK], F32, tag="xs")
                nc.sync.dma_start(
                    xs[:], xt_d[128 * kb:128 * (kb + 1),
                                M_CHUNK * mc:M_CHUNK * (mc + 1)])
                if kb < KF:
                    j = kb // 2
                    if kb % 2 == 0:
                        t8 = xwin.tile([128, 2, M_CHUNK], FP8,
                                       tag=f"x8_{j}", name=f"x8_{j}_{mc}")
                        tiles8[j] = t8
                    dst = tiles8[kb // 2][:, kb % 2, :]
                else:
                    tb = xwin.tile([128, M_CHUNK], BF16,
                                   tag=f"xb_{kb}", name=f"xb_{kb}_{mc}")
                    tilesb[kb] = tb
                    dst = tb[:]
                if kb % 2 == 0:
                    nc.scalar.mul(dst, xs[:], sh_col[:])
                else:
                    nc.vector.tensor_scalar(
                        dst, xs[:], sh_col[:], None, mybir.AluOpType.mult)

            def load_x_group(mc, inject=None):
                xg = ({}, {})
                for kb in range(KO):
                    if inject and kb in inject:
                        quant_half(*inject[kb])
                    emit_x_chunk(mc, kb, xg)
                return xg

            def chain(nb, mc, xg):
                tiles8, tilesb = xg
                ps = psum_pool.tile([128, M_CHUNK], F32, tag="ps",
                                    name=f"ps{nb}_{mc}")
                for j in range(N_PAIR):
                    nc.tensor.matmul(
                        ps[:], lhsT=qt8[nb][:, 2 * j:2 * j + 2, :],
                        rhs=tiles8[j][:],
                        start=(j == 0), stop=False,
                        perf_mode=mybir.MatmulPerfMode.DoubleRow)
                for i, kb in enumerate(range(KF, KO)):
                    nc.tensor.matmul(
                        ps[:], lhsT=qtb[nb][:, i, :], rhs=tilesb[kb][:],
                        start=(KF == 0 and i == 0), stop=(kb == KO - 1))
                ob = out_pool.tile([128, M_CHUNK], F32, tag="outp",
                                   name=f"ob{nb}_{mc}")
                nc.scalar.copy(ob[:], ps[:])
                nc.scalar.dma_start(
                    outT[128 * nb:128 * (nb + 1),
                         M_CHUNK * mc:M_CHUNK * (mc + 1)], ob[:])

            quant_half(0, 0)
            xg0 = load_x_group(0, inject={
                2: (0, 1), 5: (1, 0), 8: (1, 1), 12: (2, 0), 16: (2, 1),
                20: (3, 0), 24: (3, 1), 28: (4, 0)})
            xg1 = load_x_group(1, inject={
                0: (4, 1), 4: (5, 0), 8: (5, 1), 12: (6, 0), 16: (6, 1),
                20: (7, 0), 24: (7, 1), 28: (8, 0)})
            for nb in range(8):
                if nb < 7:
                    quant_half(nb + 8, 1)
                    quant_half(nb + 9, 0)
                chain(nb, 0, xg0)
                chain(nb, 1, xg1)
            quant_half(15, 1)
            for nb in range(8, N_NB):
                chain(nb, 0, xg0)
            xg2 = load_x_group(2)
            for nb in range(8, N_NB):
                chain(nb, 1, xg1)
            xg3 = load_x_group(3)
            for nb in range(N_NB):
                chain(nb, 2, xg2)
            for nb in range(N_NB):
                chain(nb, 3, xg3)

    nc.compile()
    return nc


def kernel(x, weight, bias):
    global LAST_RESULTS
    x = np.asarray(x, dtype=np.float32)
    weight = np.ascontiguousarray(np.asarray(weight, dtype=np.float32))
    bias = np.ascontiguousarray(np.asarray(bias, dtype=np.float32))

    if "nc_scale" not in _CACHE:
        _CACHE["nc_scale"] = _build_scale()
        _CACHE["nc_main"] = _build_main()
    nc_scale, nc_main = _CACHE["nc_scale"], _CACHE["nc_main"]

    trace = bool(int(os.environ.get("KERNEL_TRACE", "0")))
    kw = {"trace": True, "trace_cores": [0]} if trace else {}

    in_a = [{"wredN": weight[WRED * c:WRED * (c + 1)]}
            for c in range(N_CORES)]
    res_a = run_bass_kernel_spmd(nc_scale, in_a, list(range(N_CORES)), **kw)
    partials = np.array(
        [res_a.results[c]["partial"][0, 0] for c in range(N_CORES)],
        dtype=np.float32)

    xr = x.reshape(M_ALL, K)
    in_b = []
    for c in range(N_CORES):
        i, j = c // F_GRP, c % F_GRP
        w_sh = weight[N_SH * j:N_SH * (j + 1)]
        wt5 = np.ascontiguousarray(
            w_sh.reshape(N_NB, 128, KO, 128).transpose(0, 3, 2, 1))
        in_b.append({
            "xt_sh": np.ascontiguousarray(xr[M_SH * i:M_SH * (i + 1)].T),
            "wt5": wt5,
            "partials": partials,
        })
    res_b = run_bass_kernel_spmd(nc_main, in_b, list(range(N_CORES)), **kw)
    LAST_RESULTS = (res_a, res_b)

    out = np.empty((M_ALL, N_OUT), dtype=np.float32)
    for c in range(N_CORES):
        i, j = c // F_GRP, c % F_GRP
        out[M_SH * i:M_SH * (i + 1), N_SH * j:N_SH * (j + 1)] = \
            res_b.results[c]["outT"].T
    if bias.any():
        out += bias[None, :]
    return out.reshape(B, S, N_OUT)


# revision 19
# speedup vs baseline: 1.4223x; 1.0610x over previous
import os

import numpy as np

import concourse.bass as bass
import concourse.tile as tile
from concourse import bacc, mybir
from concourse.bass_utils import run_bass_kernel_spmd

N_CORES = 8
R_GRP, F_GRP = 4, 2
B, S, K = 4, 2048, 4096
N_OUT = 4096
M_ALL = B * S
M_SH = M_ALL // R_GRP
N_SH = N_OUT // F_GRP
WRED = N_OUT // N_CORES
KO = K // 128
M_CHUNK = 512
N_MC = M_SH // M_CHUNK
N_NB = N_SH // 128

N_PAIR = 10
KF = 2 * N_PAIR

C_ABS = float(np.float32(0.79788456))
THRESH = 2.0 / 3.0
F32 = mybir.dt.float32
BF16 = mybir.dt.bfloat16
FP8 = mybir.dt.float8e4

_CACHE = {}
LAST_RESULTS = None


def _build_scale():
    nc = bacc.Bacc(None, target_bir_lowering=False, num_devices=N_CORES)
    wred_d = nc.dram_tensor("wredN", [WRED, K], F32, kind="ExternalInput")
    part_d = nc.dram_tensor("partial", [1, 1], F32, kind="ExternalOutput")

    with tile.TileContext(nc) as tc:
        with (
            tc.tile_pool(name="misc", bufs=1) as misc,
            tc.tile_pool(name="redstage", bufs=4) as redstage,
            tc.tile_pool(name="psum_s", bufs=1, space="PSUM") as psum_s_pool,
        ):
            racc = misc.tile([128, 8], F32)
            for t in range(8):
                wf = redstage.tile([128, K // 2], F32, tag="redstage")
                (nc.sync if t % 2 == 0 else nc.scalar).dma_start(
                    wf[:], wred_d.rearrange("(a p) (b c) -> p a b c", p=128, b=2)
                    [:, t // 2, t % 2, :])
                if t % 2 == 0:
                    nc.vector.tensor_reduce(
                        racc[:, t:t + 1], wf[:],
                        axis=mybir.AxisListType.X, op=mybir.AluOpType.add,
                        apply_absolute_value=True)
                else:
                    nc.scalar.activation(
                        wf[:], wf[:], mybir.ActivationFunctionType.Abs,
                        accum_out=racc[:, t:t + 1])
            rsm = misc.tile([128, 8], F32)
            nc.vector.tensor_scalar(
                rsm[:], racc[:], -float(np.float32((K // 2) * np.float32(C_ABS))),
                None, mybir.AluOpType.add)
            r1 = misc.tile([128, 1], F32)
            nc.vector.tensor_reduce(
                r1[:], rsm[:], axis=mybir.AxisListType.X, op=mybir.AluOpType.add)
            ones_col = misc.tile([128, 1], F32)
            nc.vector.memset(ones_col[:], 1.0)
            ps1 = psum_s_pool.tile([1, 1], F32)
            nc.tensor.matmul(ps1[:], lhsT=r1[:], rhs=ones_col[:])
            sc = misc.tile([1, 1], F32)
            nc.vector.tensor_copy(sc[:], ps1[:])
            nc.sync.dma_start(part_d[:], sc[:])

    nc.compile()
    return nc


def _build_main():
    nc = bacc.Bacc(None, target_bir_lowering=False, num_devices=N_CORES)
    xt_d = nc.dram_tensor("xt_sh", [K, M_SH], F32, kind="ExternalInput")
    wt5_d = nc.dram_tensor("wt5", [N_NB, 128, KO, 128], F32, kind="ExternalInput")
    part_d = nc.dram_tensor("partials", [N_CORES], F32, kind="ExternalInput")
    outT = nc.dram_tensor("outT", [N_SH, M_SH], F32, kind="ExternalOutput")

    with tile.TileContext(nc) as tc:
        with (
            tc.tile_pool(name="misc", bufs=1) as misc,
            tc.tile_pool(name="xwin", bufs=2) as xwin,
            tc.tile_pool(name="xstage", bufs=4) as xstage,
            tc.tile_pool(name="wq", bufs=3) as wq_pool,
            tc.tile_pool(name="yq", bufs=3) as yq_pool,
            tc.tile_pool(name="qt", bufs=1) as qt_pool,
            tc.tile_pool(name="outp", bufs=3) as out_pool,
            tc.tile_pool(name="psum", bufs=7, space="PSUM") as psum_pool,
            tc.tile_pool(name="psum_s", bufs=1, space="PSUM") as psum_s_pool,
        ):
            pt = misc.tile([1, N_CORES], F32)
            nc.sync.dma_start(pt[:], part_d.rearrange("(p o) -> p o", p=1))
            s0 = misc.tile([1, 1], F32)
            nc.vector.tensor_reduce(
                s0[:], pt[:], axis=mybir.AxisListType.X, op=mybir.AluOpType.add)
            ones_row = misc.tile([1, 128], F32)
            nc.vector.memset(ones_row[:], 1.0)
            ps_bc = psum_s_pool.tile([128, 1], F32)
            nc.tensor.matmul(ps_bc[:], lhsT=ones_row[:], rhs=s0[:])
            mean_col = misc.tile([128, 1], F32)
            nc.vector.tensor_scalar(
                mean_col[:], ps_bc[:], 1.0 / (N_OUT * K), C_ABS,
                mybir.AluOpType.mult, mybir.AluOpType.add)
            s_col = misc.tile([128, 1], F32)
            nc.vector.tensor_scalar(
                s_col[:], mean_col[:], 1e-5, 1000.0,
                mybir.AluOpType.max, mybir.AluOpType.min)
            thr_col = misc.tile([128, 1], F32)
            nc.vector.tensor_scalar(
                thr_col[:], s_col[:], THRESH, None, mybir.AluOpType.mult)
            nthr_col = misc.tile([128, 1], F32)
            nc.vector.tensor_scalar(
                nthr_col[:], s_col[:], -THRESH, None, mybir.AluOpType.mult)
            sh_col = misc.tile([128, 1], F32)
            nc.vector.tensor_scalar(
                sh_col[:], s_col[:], 0.5, None, mybir.AluOpType.mult)

            qt8 = [qt_pool.tile([128, KF, 128], FP8, name=f"qt8_{nb}")
                   for nb in range(N_NB)] if KF else None
            qtb = [qt_pool.tile([128, KO - KF, 128], BF16, name=f"qtb_{nb}")
                   for nb in range(N_NB)]

            def quant_half(nb, h):
                wq = wq_pool.tile([128, KO // 2, 128], F32, tag="wq",
                                  name=f"wq{nb}_{h}")
                nc.sync.dma_start(wq[:], wt5_d[nb, :, 16 * h:16 * (h + 1), :])
                wq_f = wq[:].rearrange("p a b -> p (a b)")
                if h == 0:
                    sp = yq_pool.tile([128, 2048], BF16, tag="yq",
                                      name=f"sp{nb}")
                    nc.scalar.activation(
                        sp[:], wq_f, mybir.ActivationFunctionType.Sign,
                        bias=nthr_col[:])
                    sn = yq_pool.tile([128, 2048], BF16, tag="yq",
                                      name=f"sn{nb}")
                    nc.scalar.activation(
                        sn[:], wq_f, mybir.ActivationFunctionType.Sign,
                        bias=thr_col[:])
                    kf0 = min(KF, 16)
                    if kf0:
                        nc.vector.tensor_tensor(
                            qt8[nb][:, :kf0, :].rearrange("p a b -> p (a b)"),
                            sp[:, :kf0 * 128], sn[:, :kf0 * 128],
                            mybir.AluOpType.add)
                    if KF < 16:
                        nc.vector.tensor_tensor(
                            qtb[nb][:, :16 - KF, :].rearrange("p a b -> p (a b)"),
                            sp[:, KF * 128:], sn[:, KF * 128:],
                            mybir.AluOpType.add)
                else:
                    mpos = yq_pool.tile([128, 2048], BF16, tag="yq",
                                        name=f"mp{nb}")
                    nc.vector.tensor_scalar(
                        mpos[:], wq_f, thr_col[:], 2.0,
                        mybir.AluOpType.is_gt, mybir.AluOpType.mult)
                    mneg = yq_pool.tile([128, 2048], BF16, tag="yq",
                                        name=f"mn{nb}")
                    nc.vector.tensor_scalar(
                        mneg[:], wq_f, nthr_col[:], 2.0,
                        mybir.AluOpType.is_lt, mybir.AluOpType.mult)
                    kf1 = max(KF - 16, 0)
                    if kf1:
                        nc.vector.tensor_tensor(
                            qt8[nb][:, 16:KF, :].rearrange("p a b -> p (a b)"),
                            mpos[:, :kf1 * 128], mneg[:, :kf1 * 128],
                            mybir.AluOpType.subtract)
                    nc.vector.tensor_tensor(
                        qtb[nb][:, max(16 - KF, 0):, :]
                        .rearrange("p a b -> p (a b)"),
                        mpos[:, kf1 * 128:], mneg[:, kf1 * 128:],
                        mybir.AluOpType.subtract)

            def emit_x_chunk(mc, kb, xg):
                tiles8, tilesb = xg
                xs = xstage.tile([128, M_CHUNK], F32, tag="xs")
                nc.sync.dma_start(
                    xs[:], xt_d[128 * kb:128 * (kb + 1),
                                M_CHUNK * mc:M_CHUNK * (mc + 1)])
                if kb < KF:
                    j = kb // 2
                    if kb % 2 == 0:
                        t8 = xwin.tile([128, 2, M_CHUNK], FP8,
                                       tag=f"x8_{j}", name=f"x8_{j}_{mc}")
                        tiles8[j] = t8
                    dst = tiles8[kb // 2][:, kb % 2, :]
                else:
                    tb = xwin.tile([128, M_CHUNK], BF16,
                                   tag=f"xb_{kb}", name=f"xb_{kb}_{mc}")
                    tilesb[kb] = tb
                    dst = tb[:]
                if kb % 2 == 0:
                    nc.scalar.mul(dst, xs[:], sh_col[:])
                else:
                    nc.vector.tensor_scalar(
                        dst, xs[:], sh_col[:], None, mybir.AluOpType.mult)

            def load_x_group(mc, inject=None):
                xg = ({}, {})
                for kb in range(KO):
                    if inject and kb in inject:
                        quant_half(*inject[kb])
                    emit_x_chunk(mc, kb, xg)
                return xg

            def chain(nb, mc, xg):
                tiles8, tilesb = xg
                ps = psum_pool.tile([128, M_CHUNK], F32, tag="ps",
                                    name=f"ps{nb}_{mc}")
                for j in range(N_PAIR):
                    nc.tensor.matmul(
                        ps[:], lhsT=qt8[nb][:, 2 * j:2 * j + 2, :],
                        rhs=tiles8[j][:],
                        start=(j == 0), stop=False,
                        perf_mode=mybir.MatmulPerfMode.DoubleRow)
                for i, kb in enumerate(range(KF, KO)):
                    nc.tensor.matmul(
                        ps[:], lhsT=qtb[nb][:, i, :], rhs=tilesb[kb][:],
                        start=(KF == 0 and i == 0), stop=(kb == KO - 1))
                ob = out_pool.tile([128, M_CHUNK], F32, tag="outp",
                                   name=f"ob{nb}_{mc}")
                nc.scalar.copy(ob[:], ps[:])
                nc.scalar.dma_start(
                    outT[128 * nb:128 * (nb + 1),
                         M_CHUNK * mc:M_CHUNK * (mc + 1)], ob[:])

            quant_half(0, 0)
            quant_half(0, 1)
            xg0 = load_x_group(0, inject={
                4: (1, 0), 9: (1, 1), 14: (2, 0), 19: (2, 1), 24: (3, 0),
                28: (3, 1)})
            h_list = [(q, h) for q in range(4, N_NB) for h in (0, 1)]
            hi = 0
            xg1 = ({}, {})
            xg2 = ({}, {})
            xg3 = ({}, {})
            for i in range(8):
                for _ in range(2 if i % 2 == 0 else 1):
                    if hi < len(h_list):
                        quant_half(*h_list[hi]); hi += 1
                for c in range(4):
                    emit_x_chunk(1, 4 * i + c, xg1)
                chain(i, 0, xg0)
            for i in range(8, 16):
                for _ in range(2):
                    if hi < len(h_list):
                        quant_half(*h_list[hi]); hi += 1
                chain(i, 0, xg0)
                chain(i - 8, 1, xg1)
            for i in range(16, 24):
                for c in range(4):
                    emit_x_chunk(2, 4 * (i - 16) + c, xg2)
                chain(i - 8, 1, xg1)
            for nb in range(N_NB):
                if nb < 8:
                    for c in range(4):
                        emit_x_chunk(3, 4 * nb + c, xg3)
                chain(nb, 2, xg2)
            for nb in range(N_NB):
                chain(nb, 3, xg3)

    nc.compile()
    return nc


def _build_main():
    nc = bacc.Bacc(None, target_bir_lowering=False, num_devices=N_CORES)
    xt_d = nc.dram_tensor("xt_sh", [K, M_SH], F32, kind="ExternalInput")
    wt5_d = nc.dram_tensor("wt5", [N_NB, 128, KO, 128], F32, kind="ExternalInput")
    part_d = nc.dram_tensor("partials", [N_CORES], F32, kind="ExternalInput")
    outT = nc.dram_tensor("outT", [N_SH, M_SH], F32, kind="ExternalOutput")

    with tile.TileContext(nc) as tc:
        with (
            tc.tile_pool(name="misc", bufs=1) as misc,
            tc.tile_pool(name="xwin", bufs=2) as xwin,
            tc.tile_pool(name="xstage", bufs=4) as xstage,
            tc.tile_pool(name="wq", bufs=3) as wq_pool,
            tc.tile_pool(name="yq", bufs=3) as yq_pool,
            tc.tile_pool(name="qt", bufs=1) as qt_pool,
            tc.tile_pool(name="outp", bufs=3) as out_pool,
            tc.tile_pool(name="psum", bufs=7, space="PSUM") as psum_pool,
            tc.tile_pool(name="psum_s", bufs=1, space="PSUM") as psum_s_pool,
        ):
            pt = misc.tile([1, N_CORES], F32)
            nc.sync.dma_start(pt[:], part_d.rearrange("(p o) -> p o", p=1))
            s0 = misc.tile([1, 1], F32)
            nc.vector.tensor_reduce(
                s0[:], pt[:], axis=mybir.AxisListType.X, op=mybir.AluOpType.add)
            ones_row = misc.tile([1, 128], F32)
            nc.vector.memset(ones_row[:], 1.0)
            ps_bc = psum_s_pool.tile([128, 1], F32)
            nc.tensor.matmul(ps_bc[:], lhsT=ones_row[:], rhs=s0[:])
            mean_col = misc.tile([128, 1], F32)
            nc.vector.tensor_scalar(
                mean_col[:], ps_bc[:], 1.0 / (N_OUT * K), C_ABS,
                mybir.AluOpType.mult, mybir.AluOpType.add)
            s_col = misc.tile([128, 1], F32)
            nc.vector.tensor_scalar(
                s_col[:], mean_col[:], 1e-5, 1000.0,
                mybir.AluOpType.max, mybir.AluOpType.min)
            thr_col = misc.tile([128, 1], F32)
            nc.vector.tensor_scalar(
                thr_col[:], s_col[:], THRESH, None, mybir.AluOpType.mult)
            nthr_col = misc.tile([128, 1], F32)
            nc.vector.tensor_scalar(
                nthr_col[:], s_col[:], -THRESH, None, mybir.AluOpType.mult)
            sh_col = misc.tile([128, 1], F32)
            nc.vector.tensor_scalar(
                sh_col[:], s_col[:], 0.5, None, mybir.AluOpType.mult)

            qt8 = [qt_pool.tile([128, KF, 128], FP8, name=f"qt8_{nb}")
                   for nb in range(N_NB)] if KF else None
            qtb = [qt_pool.tile([128, KO - KF, 128], BF16, name=f"qtb_{nb}")
                   for nb in range(N_NB)]

            def quant_half(nb, h):
                wq = wq_pool.tile([128, KO // 2, 128], F32, tag="wq",
                                  name=f"wq{nb}_{h}")
                nc.sync.dma_start(wq[:], wt5_d[nb, :, 16 * h:16 * (h + 1), :])
                wq_f = wq[:].rearrange("p a b -> p (a b)")
                if h == 0:
                    sp = yq_pool.tile([128, 2048], BF16, tag="yq",
                                      name=f"sp{nb}")
                    nc.scalar.activation(
                        sp[:], wq_f, mybir.ActivationFunctionType.Sign,
                        bias=nthr_col[:])
                    sn = yq_pool.tile([128, 2048], BF16, tag="yq",
                                      name=f"sn{nb}")
                    nc.scalar.activation(
                        sn[:], wq_f, mybir.ActivationFunctionType.Sign,
                        bias=thr_col[:])
                    kf0 = min(KF, 16)
                    if kf0:
                        nc.vector.tensor_tensor(
                            qt8[nb][:, :kf0, :].rearrange("p a b -> p (a b)"),
                            sp[:, :kf0 * 128], sn[:, :kf0 * 128],
                            mybir.AluOpType.add)
                    if KF < 16:
                        nc.vector.tensor_tensor(
                            qtb[nb][:, :16 - KF, :].rearrange("p a b -> p (a b)"),
                            sp[:, KF * 128:], sn[:, KF * 128:],
                            mybir.AluOpType.add)
                else:
                    mpos = yq_pool.tile([128, 2048], BF16, tag="yq",
                                        name=f"mp{nb}")
                    nc.vector.tensor_scalar(
                        mpos[:], wq_f, thr_col[:], 2.0,
                        mybir.AluOpType.is_gt, mybir.AluOpType.mult)
                    mneg = yq_pool.tile([128, 2048], BF16, tag="yq",
                                        name=f"mn{nb}")
                    nc.vector.tensor_scalar(
                        mneg[:], wq_f, nthr_col[:], 2.0,
                        mybir.AluOpType.is_lt, mybir.AluOpType.mult)
                    kf1 = max(KF - 16, 0)
                    if kf1:
                        nc.vector.tensor_tensor(
                            qt8[nb][:, 16:KF, :].rearrange("p a b -> p (a b)"),
                            mpos[:, :kf1 * 128], mneg[:, :kf1 * 128],
                            mybir.AluOpType.subtract)
                    nc.vector.tensor_tensor(
                        qtb[nb][:, max(16 - KF, 0):, :]
                        .rearrange("p a b -> p (a b)"),
                        mpos[:, kf1 * 128:], mneg[:, kf1 * 128:],
                        mybir.AluOpType.subtract)

            def emit_x_chunk(mc, kb, xg):
                tiles8, tilesb = xg
                xs = xstage.tile([128, M_CHUNK], F32, tag="xs")
                nc.sync.dma_start(
                    xs[:], xt_d[128 * kb:128 * (kb + 1),
                                M_CHUNK * mc:M_CHUNK * (mc + 1)])
                if kb < KF:
                    j = kb // 2
                    if kb % 2 == 0:
                        t8 = xwin.tile([128, 2, M_CHUNK], FP8,
                                       tag=f"x8_{j}", name=f"x8_{j}_{mc}")
                        tiles8[j] = t8
                    dst = tiles8[kb // 2][:, kb % 2, :]
                else:
                    tb = xwin.tile([128, M_CHUNK], BF16,
                                   tag=f"xb_{kb}", name=f"xb_{kb}_{mc}")
                    tilesb[kb] = tb
                    dst = tb[:]
                if kb % 2 == 0:
                    nc.scalar.mul(dst, xs[:], sh_col[:])
                else:
                    nc.vector.tensor_scalar(
                        dst, xs[:], sh_col[:], None, mybir.AluOpType.mult)

            def load_x_group(mc, inject=None):
                xg = ({}, {})
                for kb in range(KO):
                    if inject and kb in inject:
                        quant_half(*inject[kb])
                    emit_x_chunk(mc, kb, xg)
                return xg

            def chain(nb, mc, xg):
                tiles8, tilesb = xg
                ps = psum_pool.tile([128, M_CHUNK], F32, tag="ps",
                                    name=f"ps{nb}_{mc}")
                for j in range(N_PAIR):
                    nc.tensor.matmul(
                        ps[:], lhsT=qt8[nb][:, 2 * j:2 * j + 2, :],
                        rhs=tiles8[j][:],
                        start=(j == 0), stop=False,
                        perf_mode=mybir.MatmulPerfMode.DoubleRow)
                for i, kb in enumerate(range(KF, KO)):
                    nc.tensor.matmul(
                        ps[:], lhsT=qtb[nb][:, i, :], rhs=tilesb[kb][:],
                        start=(KF == 0 and i == 0), stop=(kb == KO - 1))
                ob = out_pool.tile([128, M_CHUNK], F32, tag="outp",
                                   name=f"ob{nb}_{mc}")
                nc.scalar.copy(ob[:], ps[:])
                nc.scalar.dma_start(
                    outT[128 * nb:128 * (nb + 1),
                         M_CHUNK * mc:M_CHUNK * (mc + 1)], ob[:])

            quant_half(0, 0)
            quant_half(0, 1)
            xg0 = load_x_group(0, inject={
                4: (1, 0), 10: (1, 1), 16: (2, 0), 22: (2, 1), 28: (3, 0)})
            h_list = [(3, 1)] + [(q, h) for q in range(4, N_NB)
                                 for h in (0, 1)]
            hi = 0
            xg1 = ({}, {})
            for nb in range(N_NB):
                for _ in range(2):
                    if hi < len(h_list):
                        quant_half(*h_list[hi])
                        hi += 1
                emit_x_chunk(1, 2 * nb, xg1)
                emit_x_chunk(1, 2 * nb + 1, xg1)
                chain(nb, 0, xg0)
            xg2 = ({}, {})
            for nb in range(N_NB):
                emit_x_chunk(2, 2 * nb, xg2)
                emit_x_chunk(2, 2 * nb + 1, xg2)
                chain(nb, 1, xg1)
            xg3 = ({}, {})
            for nb in range(N_NB):
                emit_x_chunk(3, 2 * nb, xg3)
                emit_x_chunk(3, 2 * nb + 1, xg3)
                chain(nb, 2, xg2)
            for nb in range(N_NB):
                chain(nb, 3, xg3)

    nc.compile()
    return nc


def _build_main():
    nc = bacc.Bacc(None, target_bir_lowering=False, num_devices=N_CORES)
    xt_d = nc.dram_tensor("xt_sh", [K, M_SH], F32, kind="ExternalInput")
    wt5_d = nc.dram_tensor("wt5", [N_NB, 128, KO, 128], F32, kind="ExternalInput")
    part_d = nc.dram_tensor("partials", [N_CORES], F32, kind="ExternalInput")
    outT = nc.dram_tensor("outT", [N_SH, M_SH], F32, kind="ExternalOutput")

    with tile.TileContext(nc) as tc:
        with (
            tc.tile_pool(name="misc", bufs=1) as misc,
            tc.tile_pool(name="xwin", bufs=2) as xwin,
            tc.tile_pool(name="xstage", bufs=4) as xstage,
            tc.tile_pool(name="wq", bufs=3) as wq_pool,
            tc.tile_pool(name="yq", bufs=3) as yq_pool,
            tc.tile_pool(name="qt", bufs=1) as qt_pool,
            tc.tile_pool(name="outp", bufs=3) as out_pool,
            tc.tile_pool(name="psum", bufs=7, space="PSUM") as psum_pool,
            tc.tile_pool(name="psum_s", bufs=1, space="PSUM") as psum_s_pool,
        ):
            pt = misc.tile([1, N_CORES], F32)
            nc.sync.dma_start(pt[:], part_d.rearrange("(p o) -> p o", p=1))
            s0 = misc.tile([1, 1], F32)
            nc.vector.tensor_reduce(
                s0[:], pt[:], axis=mybir.AxisListType.X, op=mybir.AluOpType.add)
            ones_row = misc.tile([1, 128], F32)
            nc.vector.memset(ones_row[:], 1.0)
            ps_bc = psum_s_pool.tile([128, 1], F32)
            nc.tensor.matmul(ps_bc[:], lhsT=ones_row[:], rhs=s0[:])
            mean_col = misc.tile([128, 1], F32)
            nc.vector.tensor_scalar(
                mean_col[:], ps_bc[:], 1.0 / (N_OUT * K), C_ABS,
                mybir.AluOpType.mult, mybir.AluOpType.add)
            s_col = misc.tile([128, 1], F32)
            nc.vector.tensor_scalar(
                s_col[:], mean_col[:], 1e-5, 1000.0,
                mybir.AluOpType.max, mybir.AluOpType.min)
            thr_col = misc.tile([128, 1], F32)
            nc.vector.tensor_scalar(
                thr_col[:], s_col[:], THRESH, None, mybir.AluOpType.mult)
            nthr_col = misc.tile([128, 1], F32)
            nc.vector.tensor_scalar(
                nthr_col[:], s_col[:], -THRESH, None, mybir.AluOpType.mult)
            sh_col = misc.tile([128, 1], F32)
            nc.vector.tensor_scalar(
                sh_col[:], s_col[:], 0.5, None, mybir.AluOpType.mult)

            qt8 = [qt_pool.tile([128, KF, 128], FP8, name=f"qt8_{nb}")
                   for nb in range(N_NB)] if KF else None
            qtb = [qt_pool.tile([128, KO - KF, 128], BF16, name=f"qtb_{nb}")
                   for nb in range(N_NB)]

            def quant_half(nb, h):
                wq = wq_pool.tile([128, KO // 2, 128], F32, tag="wq",
                                  name=f"wq{nb}_{h}")
                nc.sync.dma_start(wq[:], wt5_d[nb, :, 16 * h:16 * (h + 1), :])
                wq_f = wq[:].rearrange("p a b -> p (a b)")
                if h == 0:
                    sp = yq_pool.tile([128, 2048], BF16, tag="yq",
                                      name=f"sp{nb}")
                    nc.scalar.activation(
                        sp[:], wq_f, mybir.ActivationFunctionType.Sign,
                        bias=nthr_col[:])
                    sn = yq_pool.tile([128, 2048], BF16, tag="yq",
                                      name=f"sn{nb}")
                    nc.scalar.activation(
                        sn[:], wq_f, mybir.ActivationFunctionType.Sign,
                        bias=thr_col[:])
                    kf0 = min(KF, 16)
                    if kf0:
                        nc.vector.tensor_tensor(
                            qt8[nb][:, :kf0, :].rearrange("p a b -> p (a b)"),
                            sp[:, :kf0 * 128], sn[:, :kf0 * 128],
                            mybir.AluOpType.add)
                    if KF < 16:
                        nc.vector.tensor_tensor(
                            qtb[nb][:, :16 - KF, :].rearrange("p a b -> p (a b)"),
                            sp[:, KF * 128:], sn[:, KF * 128:],
                            mybir.AluOpType.add)
                else:
                    mpos = yq_pool.tile([128, 2048], BF16, tag="yq",
                                        name=f"mp{nb}")
                    nc.vector.tensor_scalar(
                        mpos[:], wq_f, thr_col[:], 2.0,
                        mybir.AluOpType.is_gt, mybir.AluOpType.mult)
                    mneg = yq_pool.tile([128, 2048], BF16, tag="yq",
                                        name=f"mn{nb}")
                    nc.vector.tensor_scalar(
                        mneg[:], wq_f, nthr_col[:], 2.0,
                        mybir.AluOpType.is_lt, mybir.AluOpType.mult)
                    kf1 = max(KF - 16, 0)
                    if kf1:
                        nc.vector.tensor_tensor(
                            qt8[nb][:, 16:KF, :].rearrange("p a b -> p (a b)"),
                            mpos[:, :kf1 * 128], mneg[:, :kf1 * 128],
                            mybir.AluOpType.subtract)
                    nc.vector.tensor_tensor(
                        qtb[nb][:, max(16 - KF, 0):, :]
                        .rearrange("p a b -> p (a b)"),
                        mpos[:, kf1 * 128:], mneg[:, kf1 * 128:],
                        mybir.AluOpType.subtract)

            def emit_x_chunk(mc, kb, xg):
                tiles8, tilesb = xg
                xs = xstage.tile([128, M_CHUNK], F32, tag="xs")
                nc.sync.dma_start(
                    xs[:], xt_d[128 * kb:128 * (kb + 1),
                                M_CHUNK * mc:M_CHUNK * (mc + 1)])
                if kb < KF:
                    j = kb // 2
                    if kb % 2 == 0:
                        t8 = xwin.tile([128, 2, M_CHUNK], FP8,
                                       tag=f"x8_{j}", name=f"x8_{j}_{mc}")
                        tiles8[j] = t8
                    dst = tiles8[kb // 2][:, kb % 2, :]
                else:
                    tb = xwin.tile([128, M_CHUNK], BF16,
                                   tag=f"xb_{kb}", name=f"xb_{kb}_{mc}")
                    tilesb[kb] = tb
                    dst = tb[:]
                if kb % 2 == 0:
                    nc.scalar.mul(dst, xs[:], sh_col[:])
                else:
                    nc.vector.tensor_scalar(
                        dst, xs[:], sh_col[:], None, mybir.AluOpType.mult)

            def load_x_group(mc, inject=None):
                xg = ({}, {})
                for kb in range(KO):
                    if inject and kb in inject:
                        quant_half(*inject[kb])
                    emit_x_chunk(mc, kb, xg)
                return xg

            def chain(nb, mc, xg):
                tiles8, tilesb = xg
                ps = psum_pool.tile([128, M_CHUNK], F32, tag="ps",
                                    name=f"ps{nb}_{mc}")
                for j in range(N_PAIR):
                    nc.tensor.matmul(
                        ps[:], lhsT=qt8[nb][:, 2 * j:2 * j + 2, :],
                        rhs=tiles8[j][:],
                        start=(j == 0), stop=False,
                        perf_mode=mybir.MatmulPerfMode.DoubleRow)
                for i, kb in enumerate(range(KF, KO)):
                    nc.tensor.matmul(
                        ps[:], lhsT=qtb[nb][:, i, :], rhs=tilesb[kb][:],
                        start=(KF == 0 and i == 0), stop=(kb == KO - 1))
                ob = out_pool.tile([128, M_CHUNK], F32, tag="outp",
                                   name=f"ob{nb}_{mc}")
                nc.scalar.copy(ob[:], ps[:])
                nc.scalar.dma_start(
                    outT[128 * nb:128 * (nb + 1),
                         M_CHUNK * mc:M_CHUNK * (mc + 1)], ob[:])

            quant_half(0, 0)
            xg0 = load_x_group(0, inject={
                2: (0, 1), 5: (1, 0), 8: (1, 1), 12: (2, 0), 16: (2, 1),
                20: (3, 0), 24: (3, 1), 28: (4, 0)})
            xg1 = load_x_group(1, inject={
                0: (4, 1), 4: (5, 0), 8: (5, 1), 12: (6, 0), 16: (6, 1),
                20: (7, 0), 24: (7, 1), 28: (8, 0)})
            for nb in range(8):
                if nb < 7:
                    quant_half(nb + 8, 1)
                    quant_half(nb + 9, 0)
                chain(nb, 0, xg0)
                chain(nb, 1, xg1)
            quant_half(15, 1)
            for nb in range(8, N_NB):
                chain(nb, 0, xg0)
            xg2 = load_x_group(2)
            for nb in range(8, N_NB):
                chain(nb, 1, xg1)
            xg3 = load_x_group(3)
            for nb in range(N_NB):
                chain(nb, 2, xg2)
            for nb in range(N_NB):
                chain(nb, 3, xg3)

    nc.compile()
    return nc


def kernel(x, weight, bias):
    global LAST_RESULTS
    x = np.asarray(x, dtype=np.float32)
    weight = np.ascontiguousarray(np.asarray(weight, dtype=np.float32))
    bias = np.ascontiguousarray(np.asarray(bias, dtype=np.float32))

    if "nc_scale" not in _CACHE:
        _CACHE["nc_scale"] = _build_scale()
        _CACHE["nc_main"] = _build_main()
    nc_scale, nc_main = _CACHE["nc_scale"], _CACHE["nc_main"]

    trace = bool(int(os.environ.get("KERNEL_TRACE", "0")))
    kw = {"trace": True, "trace_cores": [0]} if trace else {}

    in_a = [{"wredN": weight[WRED * c:WRED * (c + 1)]}
            for c in range(N_CORES)]
    res_a = run_bass_kernel_spmd(nc_scale, in_a, list(range(N_CORES)), **kw)
    partials = np.array(
        [res_a.results[c]["partial"][0, 0] for c in range(N_CORES)],
        dtype=np.float32)

    xr = x.reshape(M_ALL, K)
    in_b = []
    for c in range(N_CORES):
        i, j = c // F_GRP, c % F_GRP
        w_sh = weight[N_SH * j:N_SH * (j + 1)]
        wt5 = np.ascontiguousarray(
            w_sh.reshape(N_NB, 128, KO, 128).transpose(0, 3, 2, 1))
        in_b.append({
            "xt_sh": np.ascontiguousarray(xr[M_SH * i:M_SH * (i + 1)].T),
            "wt5": wt5,
            "partials": partials,
        })
    res_b = run_bass_kernel_spmd(nc_main, in_b, list(range(N_CORES)), **kw)
    LAST_RESULTS = (res_a, res_b)

    out = np.empty((M_ALL, N_OUT), dtype=np.float32)
    for c in range(N_CORES):
        i, j = c // F_GRP, c % F_GRP
        out[M_SH * i:M_SH * (i + 1), N_SH * j:N_SH * (j + 1)] = \
            res_b.results[c]["outT"].T
    if bias.any():
        out += bias[None, :]
    return out.reshape(B, S, N_OUT)


# revision 20
# speedup vs baseline: 1.4264x; 1.0029x over previous
import os

import numpy as np

import concourse.bass as bass
import concourse.tile as tile
from concourse import bacc, mybir
from concourse.bass_utils import run_bass_kernel_spmd

N_CORES = 8
R_GRP, F_GRP = 4, 2
B, S, K = 4, 2048, 4096
N_OUT = 4096
M_ALL = B * S
M_SH = M_ALL // R_GRP
N_SH = N_OUT // F_GRP
WRED = N_OUT // N_CORES
KO = K // 128
M_CHUNK = 512
N_MC = M_SH // M_CHUNK
N_NB = N_SH // 128

N_PAIR = 10
KF = 2 * N_PAIR

C_ABS = float(np.float32(0.79788456))
THRESH = 2.0 / 3.0
F32 = mybir.dt.float32
BF16 = mybir.dt.bfloat16
FP8 = mybir.dt.float8e4

_CACHE = {}
LAST_RESULTS = None


def _build_scale():
    nc = bacc.Bacc(None, target_bir_lowering=False, num_devices=N_CORES)
    wred_d = nc.dram_tensor("wredN", [WRED, K], F32, kind="ExternalInput")
    part_d = nc.dram_tensor("partial", [1, 1], F32, kind="ExternalOutput")

    with tile.TileContext(nc) as tc:
        with (
            tc.tile_pool(name="misc", bufs=1) as misc,
            tc.tile_pool(name="redstage", bufs=4) as redstage,
            tc.tile_pool(name="psum_s", bufs=1, space="PSUM") as psum_s_pool,
        ):
            racc = misc.tile([128, 8], F32)
            for t in range(8):
                wf = redstage.tile([128, K // 2], F32, tag="redstage")
                (nc.sync if t % 2 == 0 else nc.scalar).dma_start(
                    wf[:], wred_d.rearrange("(a p) (b c) -> p a b c", p=128, b=2)
                    [:, t // 2, t % 2, :])
                if t % 2 == 0:
                    nc.vector.tensor_reduce(
                        racc[:, t:t + 1], wf[:],
                        axis=mybir.AxisListType.X, op=mybir.AluOpType.add,
                        apply_absolute_value=True)
                else:
                    nc.scalar.activation(
                        wf[:], wf[:], mybir.ActivationFunctionType.Abs,
                        accum_out=racc[:, t:t + 1])
            rsm = misc.tile([128, 8], F32)
            nc.vector.tensor_scalar(
                rsm[:], racc[:], -float(np.float32((K // 2) * np.float32(C_ABS))),
                None, mybir.AluOpType.add)
            r1 = misc.tile([128, 1], F32)
            nc.vector.tensor_reduce(
                r1[:], rsm[:], axis=mybir.AxisListType.X, op=mybir.AluOpType.add)
            ones_col = misc.tile([128, 1], F32)
            nc.vector.memset(ones_col[:], 1.0)
            ps1 = psum_s_pool.tile([1, 1], F32)
            nc.tensor.matmul(ps1[:], lhsT=r1[:], rhs=ones_col[:])
            sc = misc.tile([1, 1], F32)
            nc.vector.tensor_copy(sc[:], ps1[:])
            nc.sync.dma_start(part_d[:], sc[:])

    nc.compile()
    return nc


def _build_main():
    nc = bacc.Bacc(None, target_bir_lowering=False, num_devices=N_CORES)
    xt_d = nc.dram_tensor("xt_sh", [K, M_SH], F32, kind="ExternalInput")
    wt5_d = nc.dram_tensor("wt5", [N_NB, 128, KO, 128], F32, kind="ExternalInput")
    part_d = nc.dram_tensor("partials", [N_CORES], F32, kind="ExternalInput")
    outT = nc.dram_tensor("outT", [N_SH, M_SH], F32, kind="ExternalOutput")

    with tile.TileContext(nc) as tc:
        with (
            tc.tile_pool(name="misc", bufs=1) as misc,
            tc.tile_pool(name="xwin", bufs=2) as xwin,
            tc.tile_pool(name="xstage", bufs=4) as xstage,
            tc.tile_pool(name="wq", bufs=3) as wq_pool,
            tc.tile_pool(name="yq", bufs=3) as yq_pool,
            tc.tile_pool(name="qt", bufs=1) as qt_pool,
            tc.tile_pool(name="outp", bufs=3) as out_pool,
            tc.tile_pool(name="psum", bufs=7, space="PSUM") as psum_pool,
            tc.tile_pool(name="psum_s", bufs=1, space="PSUM") as psum_s_pool,
        ):
            pt = misc.tile([1, N_CORES], F32)
            nc.sync.dma_start(pt[:], part_d.rearrange("(p o) -> p o", p=1))
            s0 = misc.tile([1, 1], F32)
            nc.vector.tensor_reduce(
                s0[:], pt[:], axis=mybir.AxisListType.X, op=mybir.AluOpType.add)
            ones_row = misc.tile([1, 128], F32)
            nc.vector.memset(ones_row[:], 1.0)
            ps_bc = psum_s_pool.tile([128, 1], F32)
            nc.tensor.matmul(ps_bc[:], lhsT=ones_row[:], rhs=s0[:])
            mean_col = misc.tile([128, 1], F32)
            nc.vector.tensor_scalar(
                mean_col[:], ps_bc[:], 1.0 / (N_OUT * K), C_ABS,
                mybir.AluOpType.mult, mybir.AluOpType.add)
            s_col = misc.tile([128, 1], F32)
            nc.vector.tensor_scalar(
                s_col[:], mean_col[:], 1e-5, 1000.0,
                mybir.AluOpType.max, mybir.AluOpType.min)
            thr_col = misc.tile([128, 1], F32)
            nc.vector.tensor_scalar(
                thr_col[:], s_col[:], THRESH, None, mybir.AluOpType.mult)
            nthr_col = misc.tile([128, 1], F32)
            nc.vector.tensor_scalar(
                nthr_col[:], s_col[:], -THRESH, None, mybir.AluOpType.mult)
            sh_col = misc.tile([128, 1], F32)
            nc.vector.tensor_scalar(
                sh_col[:], s_col[:], 0.5, None, mybir.AluOpType.mult)

            qt8 = [qt_pool.tile([128, KF, 128], FP8, name=f"qt8_{nb}")
                   for nb in range(N_NB)] if KF else None
            qtb = [qt_pool.tile([128, KO - KF, 128], BF16, name=f"qtb_{nb}")
                   for nb in range(N_NB)]

            def quant_half(nb, h):
                wq = wq_pool.tile([128, KO // 2, 128], F32, tag="wq",
                                  name=f"wq{nb}_{h}")
                nc.sync.dma_start(wq[:], wt5_d[nb, :, 16 * h:16 * (h + 1), :])
                wq_f = wq[:].rearrange("p a b -> p (a b)")
                if h == 0:
                    sp = yq_pool.tile([128, 2048], BF16, tag="yq",
                                      name=f"sp{nb}")
                    nc.scalar.activation(
                        sp[:], wq_f, mybir.ActivationFunctionType.Sign,
                        bias=nthr_col[:])
                    sn = yq_pool.tile([128, 2048], BF16, tag="yq",
                                      name=f"sn{nb}")
                    nc.scalar.activation(
                        sn[:], wq_f, mybir.ActivationFunctionType.Sign,
                        bias=thr_col[:])
                    kf0 = min(KF, 16)
                    if kf0:
                        nc.vector.tensor_tensor(
                            qt8[nb][:, :kf0, :].rearrange("p a b -> p (a b)"),
                            sp[:, :kf0 * 128], sn[:, :kf0 * 128],
                            mybir.AluOpType.add)
                    if KF < 16:
                        nc.vector.tensor_tensor(
                            qtb[nb][:, :16 - KF, :].rearrange("p a b -> p (a b)"),
                            sp[:, KF * 128:], sn[:, KF * 128:],
                            mybir.AluOpType.add)
                else:
                    mpos = yq_pool.tile([128, 2048], BF16, tag="yq",
                                        name=f"mp{nb}")
                    nc.vector.tensor_scalar(
                        mpos[:], wq_f, thr_col[:], 2.0,
                        mybir.AluOpType.is_gt, mybir.AluOpType.mult)
                    mneg = yq_pool.tile([128, 2048], BF16, tag="yq",
                                        name=f"mn{nb}")
                    nc.vector.tensor_scalar(
                        mneg[:], wq_f, nthr_col[:], 2.0,
                        mybir.AluOpType.is_lt, mybir.AluOpType.mult)
                    kf1 = max(KF - 16, 0)
                    if kf1:
                        nc.vector.tensor_tensor(
                            qt8[nb][:, 16:KF, :].rearrange("p a b -> p (a b)"),
                            mpos[:, :kf1 * 128], mneg[:, :kf1 * 128],
                            mybir.AluOpType.subtract)
                    nc.vector.tensor_tensor(
                        qtb[nb][:, max(16 - KF, 0):, :]
                        .rearrange("p a b -> p (a b)"),
                        mpos[:, kf1 * 128:], mneg[:, kf1 * 128:],
                        mybir.AluOpType.subtract)

            def emit_x_chunk(mc, kb, xg):
                tiles8, tilesb = xg
                xs = xstage.tile([128, M_CHUNK], F32, tag="xs")
                nc.sync.dma_start(
                    xs[:], xt_d[128 * kb:128 * (kb + 1),
                                M_CHUNK * mc:M_CHUNK * (mc + 1)])
                if kb < KF:
                    j = kb // 2
                    if kb % 2 == 0:
                        t8 = xwin.tile([128, 2, M_CHUNK], FP8,
                                       tag=f"x8_{j}", name=f"x8_{j}_{mc}")
                        tiles8[j] = t8
                    dst = tiles8[kb // 2][:, kb % 2, :]
                else:
                    tb = xwin.tile([128, M_CHUNK], BF16,
                                   tag=f"xb_{kb}", name=f"xb_{kb}_{mc}")
                    tilesb[kb] = tb
                    dst = tb[:]
                if kb % 2 == 0:
                    nc.scalar.mul(dst, xs[:], sh_col[:])
                else:
                    nc.vector.tensor_scalar(
                        dst, xs[:], sh_col[:], None, mybir.AluOpType.mult)

            def load_x_group(mc, inject=None):
                xg = ({}, {})
                for kb in range(KO):
                    if inject and kb in inject:
                        quant_half(*inject[kb])
                    emit_x_chunk(mc, kb, xg)
                return xg

            def chain(nb, mc, xg):
                tiles8, tilesb = xg
                ps = psum_pool.tile([128, M_CHUNK], F32, tag="ps",
                                    name=f"ps{nb}_{mc}")
                for j in range(N_PAIR):
                    nc.tensor.matmul(
                        ps[:], lhsT=qt8[nb][:, 2 * j:2 * j + 2, :],
                        rhs=tiles8[j][:],
                        start=(j == 0), stop=False,
                        perf_mode=mybir.MatmulPerfMode.DoubleRow)
                for i, kb in enumerate(range(KF, KO)):
                    nc.tensor.matmul(
                        ps[:], lhsT=qtb[nb][:, i, :], rhs=tilesb[kb][:],
                        start=(KF == 0 and i == 0), stop=(kb == KO - 1))
                ob = out_pool.tile([128, M_CHUNK], F32, tag="outp",
                                   name=f"ob{nb}_{mc}")
                nc.scalar.copy(ob[:], ps[:])
                nc.scalar.dma_start(
                    outT[128 * nb:128 * (nb + 1),
                         M_CHUNK * mc:M_CHUNK * (mc + 1)], ob[:])

            quant_half(0, 0)
            quant_half(0, 1)
            xg0 = load_x_group(0, inject={
                4: (1, 0), 9: (1, 1), 14: (2, 0), 19: (2, 1), 24: (3, 0),
                28: (3, 1)})
            h_list = [(q, h) for q in range(4, N_NB) for h in (0, 1)]
            hi = 0
            xg1 = ({}, {})
            xg2 = ({}, {})
            xg3 = ({}, {})
            for i in range(8):
                for _ in range(2 if i % 2 == 0 else 1):
                    if hi < len(h_list):
                        quant_half(*h_list[hi]); hi += 1
                for c in range(4):
                    emit_x_chunk(1, 4 * i + c, xg1)
                chain(i, 0, xg0)
            for i in range(8, 16):
                for _ in range(3):
                    if hi < len(h_list):
                        quant_half(*h_list[hi]); hi += 1
                chain(i, 0, xg0)
                chain(i - 8, 1, xg1)
            for i in range(16, 24):
                for c in range(4):
                    emit_x_chunk(2, 4 * (i - 16) + c, xg2)
                chain(i - 8, 1, xg1)
            for nb in range(N_NB):
                if nb < 8:
                    for c in range(4):
                        emit_x_chunk(3, 4 * nb + c, xg3)
                chain(nb, 2, xg2)
            for nb in range(N_NB):
                chain(nb, 3, xg3)

    nc.compile()
    return nc


def _build_main():
    nc = bacc.Bacc(None, target_bir_lowering=False, num_devices=N_CORES)
    xt_d = nc.dram_tensor("xt_sh", [K, M_SH], F32, kind="ExternalInput")
    wt5_d = nc.dram_tensor("wt5", [N_NB, 128, KO, 128], F32, kind="ExternalInput")
    part_d = nc.dram_tensor("partials", [N_CORES], F32, kind="ExternalInput")
    outT = nc.dram_tensor("outT", [N_SH, M_SH], F32, kind="ExternalOutput")

    with tile.TileContext(nc) as tc:
        with (
            tc.tile_pool(name="misc", bufs=1) as misc,
            tc.tile_pool(name="xwin", bufs=2) as xwin,
            tc.tile_pool(name="xstage", bufs=4) as xstage,
            tc.tile_pool(name="wq", bufs=3) as wq_pool,
            tc.tile_pool(name="yq", bufs=3) as yq_pool,
            tc.tile_pool(name="qt", bufs=1) as qt_pool,
            tc.tile_pool(name="outp", bufs=3) as out_pool,
            tc.tile_pool(name="psum", bufs=7, space="PSUM") as psum_pool,
            tc.tile_pool(name="psum_s", bufs=1, space="PSUM") as psum_s_pool,
        ):
            pt = misc.tile([1, N_CORES], F32)
            nc.sync.dma_start(pt[:], part_d.rearrange("(p o) -> p o", p=1))
            s0 = misc.tile([1, 1], F32)
            nc.vector.tensor_reduce(
                s0[:], pt[:], axis=mybir.AxisListType.X, op=mybir.AluOpType.add)
            ones_row = misc.tile([1, 128], F32)
            nc.vector.memset(ones_row[:], 1.0)
            ps_bc = psum_s_pool.tile([128, 1], F32)
            nc.tensor.matmul(ps_bc[:], lhsT=ones_row[:], rhs=s0[:])
            mean_col = misc.tile([128, 1], F32)
            nc.vector.tensor_scalar(
                mean_col[:], ps_bc[:], 1.0 / (N_OUT * K), C_ABS,
                mybir.AluOpType.mult, mybir.AluOpType.add)
            s_col = misc.tile([128, 1], F32)
            nc.vector.tensor_scalar(
                s_col[:], mean_col[:], 1e-5, 1000.0,
                mybir.AluOpType.max, mybir.AluOpType.min)
            thr_col = misc.tile([128, 1], F32)
            nc.vector.tensor_scalar(
                thr_col[:], s_col[:], THRESH, None, mybir.AluOpType.mult)
            nthr_col = misc.tile([128, 1], F32)
            nc.vector.tensor_scalar(
                nthr_col[:], s_col[:], -THRESH, None, mybir.AluOpType.mult)
            sh_col = misc.tile([128, 1], F32)
            nc.vector.tensor_scalar(
                sh_col[:], s_col[:], 0.5, None, mybir.AluOpType.mult)

            qt8 = [qt_pool.tile([128, KF, 128], FP8, name=f"qt8_{nb}")
                   for nb in range(N_NB)] if KF else None
            qtb = [qt_pool.tile([128, KO - KF, 128], BF16, name=f"qtb_{nb}")
                   for nb in range(N_NB)]

            def quant_half(nb, h):
                wq = wq_pool.tile([128, KO // 2, 128], F32, tag="wq",
                                  name=f"wq{nb}_{h}")
                nc.sync.dma_start(wq[:], wt5_d[nb, :, 16 * h:16 * (h + 1), :])
                wq_f = wq[:].rearrange("p a b -> p (a b)")
                if h == 0:
                    sp = yq_pool.tile([128, 2048], BF16, tag="yq",
                                      name=f"sp{nb}")
                    nc.scalar.activation(
                        sp[:], wq_f, mybir.ActivationFunctionType.Sign,
                        bias=nthr_col[:])
                    sn = yq_pool.tile([128, 2048], BF16, tag="yq",
                                      name=f"sn{nb}")
                    nc.scalar.activation(
                        sn[:], wq_f, mybir.ActivationFunctionType.Sign,
                        bias=thr_col[:])
                    kf0 = min(KF, 16)
                    if kf0:
                        nc.vector.tensor_tensor(
                            qt8[nb][:, :kf0, :].rearrange("p a b -> p (a b)"),
                            sp[:, :kf0 * 128], sn[:, :kf0 * 128],
                            mybir.AluOpType.add)
                    if KF < 16:
                        nc.vector.tensor_tensor(
                            qtb[nb][:, :16 - KF, :].rearrange("p a b -> p (a b)"),
                            sp[:, KF * 128:], sn[:, KF * 128:],
                            mybir.AluOpType.add)
                else:
                    mpos = yq_pool.tile([128, 2048], BF16, tag="yq",
                                        name=f"mp{nb}")
                    nc.vector.tensor_scalar(
                        mpos[:], wq_f, thr_col[:], 2.0,
                        mybir.AluOpType.is_gt, mybir.AluOpType.mult)
                    mneg = yq_pool.tile([128, 2048], BF16, tag="yq",
                                        name=f"mn{nb}")
                    nc.vector.tensor_scalar(
                        mneg[:], wq_f, nthr_col[:], 2.0,
                        mybir.AluOpType.is_lt, mybir.AluOpType.mult)
                    kf1 = max(KF - 16, 0)
                    if kf1:
                        nc.vector.tensor_tensor(
                            qt8[nb][:, 16:KF, :].rearrange("p a b -> p (a b)"),
                            mpos[:, :kf1 * 128], mneg[:, :kf1 * 128],
                            mybir.AluOpType.subtract)
                    nc.vector.tensor_tensor(
                        qtb[nb][:, max(16 - KF, 0):, :]
                        .rearrange("p a b -> p (a b)"),
                        mpos[:, kf1 * 128:], mneg[:, kf1 * 128:],
                        mybir.AluOpType.subtract)

            def emit_x_chunk(mc, kb, xg):
                tiles8, tilesb = xg
                xs = xstage.tile([128, M_CHUNK], F32, tag="xs")
                nc.sync.dma_start(
                    xs[:], xt_d[128 * kb:128 * (kb + 1),
                                M_CHUNK * mc:M_CHUNK * (mc + 1)])
                if kb < KF:
                    j = kb // 2
                    if kb % 2 == 0:
                        t8 = xwin.tile([128, 2, M_CHUNK], FP8,
                                       tag=f"x8_{j}", name=f"x8_{j}_{mc}")
                        tiles8[j] = t8
                    dst = tiles8[kb // 2][:, kb % 2, :]
                else:
                    tb = xwin.tile([128, M_CHUNK], BF16,
                                   tag=f"xb_{kb}", name=f"xb_{kb}_{mc}")
                    tilesb[kb] = tb
                    dst = tb[:]
                if kb % 2 == 0:
                    nc.scalar.mul(dst, xs[:], sh_col[:])
                else:
                    nc.vector.tensor_scalar(
                        dst, xs[:], sh_col[:], None, mybir.AluOpType.mult)

            def load_x_group(mc, inject=None):
                xg = ({}, {})
                for kb in range(KO):
                    if inject and kb in inject:
                        quant_half(*inject[kb])
                    emit_x_chunk(mc, kb, xg)
                return xg

            def chain(nb, mc, xg):
                tiles8, tilesb = xg
                ps = psum_pool.tile([128, M_CHUNK], F32, tag="ps",
                                    name=f"ps{nb}_{mc}")
                for j in range(N_PAIR):
                    nc.tensor.matmul(
                        ps[:], lhsT=qt8[nb][:, 2 * j:2 * j + 2, :],
                        rhs=tiles8[j][:],
                        start=(j == 0), stop=False,
                        perf_mode=mybir.MatmulPerfMode.DoubleRow)
                for i, kb in enumerate(range(KF, KO)):
                    nc.tensor.matmul(
                        ps[:], lhsT=qtb[nb][:, i, :], rhs=tilesb[kb][:],
                        start=(KF == 0 and i == 0), stop=(kb == KO - 1))
                ob = out_pool.tile([128, M_CHUNK], F32, tag="outp",
                                   name=f"ob{nb}_{mc}")
                nc.scalar.copy(ob[:], ps[:])
                nc.scalar.dma_start(
                    outT[128 * nb:128 * (nb + 1),
                         M_CHUNK * mc:M_CHUNK * (mc + 1)], ob[:])

            quant_half(0, 0)
            quant_half(0, 1)
            xg0 = load_x_group(0, inject={
                4: (1, 0), 10: (1, 1), 16: (2, 0), 22: (2, 1), 28: (3, 0)})
            h_list = [(3, 1)] + [(q, h) for q in range(4, N_NB)
                                 for h in (0, 1)]
            hi = 0
            xg1 = ({}, {})
            for nb in range(N_NB):
                for _ in range(2):
                    if hi < len(h_list):
                        quant_half(*h_list[hi])
                        hi += 1
                emit_x_chunk(1, 2 * nb, xg1)
                emit_x_chunk(1, 2 * nb + 1, xg1)
                chain(nb, 0, xg0)
            xg2 = ({}, {})
            for nb in range(N_NB):
                emit_x_chunk(2, 2 * nb, xg2)
                emit_x_chunk(2, 2 * nb + 1, xg2)
                chain(nb, 1, xg1)
            xg3 = ({}, {})
            for nb in range(N_NB):
                emit_x_chunk(3, 2 * nb, xg3)
                emit_x_chunk(3, 2 * nb + 1, xg3)
                chain(nb, 2, xg2)
            for nb in range(N_NB):
                chain(nb, 3, xg3)

    nc.compile()
    return nc


def _build_main():
    nc = bacc.Bacc(None, target_bir_lowering=False, num_devices=N_CORES)
    xt_d = nc.dram_tensor("xt_sh", [K, M_SH], F32, kind="ExternalInput")
    wt5_d = nc.dram_tensor("wt5", [N_NB, 128, KO, 128], F32, kind="ExternalInput")
    part_d = nc.dram_tensor("partials", [N_CORES], F32, kind="ExternalInput")
    outT = nc.dram_tensor("outT", [N_SH, M_SH], F32, kind="ExternalOutput")

    with tile.TileContext(nc) as tc:
        with (
            tc.tile_pool(name="misc", bufs=1) as misc,
            tc.tile_pool(name="xwin", bufs=2) as xwin,
            tc.tile_pool(name="xstage", bufs=4) as xstage,
            tc.tile_pool(name="wq", bufs=3) as wq_pool,
            tc.tile_pool(name="yq", bufs=3) as yq_pool,
            tc.tile_pool(name="qt", bufs=1) as qt_pool,
            tc.tile_pool(name="outp", bufs=3) as out_pool,
            tc.tile_pool(name="psum", bufs=7, space="PSUM") as psum_pool,
            tc.tile_pool(name="psum_s", bufs=1, space="PSUM") as psum_s_pool,
        ):
            pt = misc.tile([1, N_CORES], F32)
            nc.sync.dma_start(pt[:], part_d.rearrange("(p o) -> p o", p=1))
            s0 = misc.tile([1, 1], F32)
            nc.vector.tensor_reduce(
                s0[:], pt[:], axis=mybir.AxisListType.X, op=mybir.AluOpType.add)
            ones_row = misc.tile([1, 128], F32)
            nc.vector.memset(ones_row[:], 1.0)
            ps_bc = psum_s_pool.tile([128, 1], F32)
            nc.tensor.matmul(ps_bc[:], lhsT=ones_row[:], rhs=s0[:])
            mean_col = misc.tile([128, 1], F32)
            nc.vector.tensor_scalar(
                mean_col[:], ps_bc[:], 1.0 / (N_OUT * K), C_ABS,
                mybir.AluOpType.mult, mybir.AluOpType.add)
            s_col = misc.tile([128, 1], F32)
            nc.vector.tensor_scalar(
                s_col[:], mean_col[:], 1e-5, 1000.0,
                mybir.AluOpType.max, mybir.AluOpType.min)
            thr_col = misc.tile([128, 1], F32)
            nc.vector.tensor_scalar(
                thr_col[:], s_col[:], THRESH, None, mybir.AluOpType.mult)
            nthr_col = misc.tile([128, 1], F32)
            nc.vector.tensor_scalar(
                nthr_col[:], s_col[:], -THRESH, None, mybir.AluOpType.mult)
            sh_col = misc.tile([128, 1], F32)
            nc.vector.tensor_scalar(
                sh_col[:], s_col[:], 0.5, None, mybir.AluOpType.mult)

            qt8 = [qt_pool.tile([128, KF, 128], FP8, name=f"qt8_{nb}")
                   for nb in range(N_NB)] if KF else None
            qtb = [qt_pool.tile([128, KO - KF, 128], BF16, name=f"qtb_{nb}")
                   for nb in range(N_NB)]

            def quant_half(nb, h):
                wq = wq_pool.tile([128, KO // 2, 128], F32, tag="wq",
                                  name=f"wq{nb}_{h}")
                nc.sync.dma_start(wq[:], wt5_d[nb, :, 16 * h:16 * (h + 1), :])
                wq_f = wq[:].rearrange("p a b -> p (a b)")
                if h == 0:
                    sp = yq_pool.tile([128, 2048], BF16, tag="yq",
                                      name=f"sp{nb}")
                    nc.scalar.activation(
                        sp[:], wq_f, mybir.ActivationFunctionType.Sign,
                        bias=nthr_col[:])
                    sn = yq_pool.tile([128, 2048], BF16, tag="yq",
                                      name=f"sn{nb}")
                    nc.scalar.activation(
                        sn[:], wq_f, mybir.ActivationFunctionType.Sign,
                        bias=thr_col[:])
                    kf0 = min(KF, 16)
                    if kf0:
                        nc.vector.tensor_tensor(
                            qt8[nb][:, :kf0, :].rearrange("p a b -> p (a b)"),
                            sp[:, :kf0 * 128], sn[:, :kf0 * 128],
                            mybir.AluOpType.add)
                    if KF < 16:
                        nc.vector.tensor_tensor(
                            qtb[nb][:, :16 - KF, :].rearrange("p a b -> p (a b)"),
                            sp[:, KF * 128:], sn[:, KF * 128:],
                            mybir.AluOpType.add)
                else:
                    mpos = yq_pool.tile([128, 2048], BF16, tag="yq",
                                        name=f"mp{nb}")
                    nc.vector.tensor_scalar(
                        mpos[:], wq_f, thr_col[:], 2.0,
                        mybir.AluOpType.is_gt, mybir.AluOpType.mult)
                    mneg = yq_pool.tile([128, 2048], BF16, tag="yq",
                                        name=f"mn{nb}")
                    nc.vector.tensor_scalar(
                        mneg[:], wq_f, nthr_col[:], 2.0,
                        mybir.AluOpType.is_lt, mybir.AluOpType.mult)
                    kf1 = max(KF - 16, 0)
                    if kf1:
                        nc.vector.tensor_tensor(
                            qt8[nb][:, 16:KF, :].rearrange("p a b -> p (a b)"),
                            mpos[:, :kf1 * 128], mneg[:, :kf1 * 128],
                            mybir.AluOpType.subtract)
                    nc.vector.tensor_tensor(
                        qtb[nb][:, max(16 - KF, 0):, :]
                        .rearrange("p a b -> p (a b)"),
                        mpos[:, kf1 * 128:], mneg[:, kf1 * 128:],
                        mybir.AluOpType.subtract)

            def emit_x_chunk(mc, kb, xg):
                tiles8, tilesb = xg
                xs = xstage.tile([128, M_CHUNK], F32, tag="xs")
                nc.sync.dma_start(
                    xs[:], xt_d[128 * kb:128 * (kb + 1),
                                M_CHUNK * mc:M_CHUNK * (mc + 1)])
                if kb < KF:
                    j = kb // 2
                    if kb % 2 == 0:
                        t8 = xwin.tile([128, 2, M_CHUNK], FP8,
                                       tag=f"x8_{j}", name=f"x8_{j}_{mc}")
                        tiles8[j] = t8
                    dst = tiles8[kb // 2][:, kb % 2, :]
                else:
                    tb = xwin.tile([128, M_CHUNK], BF16,
                                   tag=f"xb_{kb}", name=f"xb_{kb}_{mc}")
                    tilesb[kb] = tb
                    dst = tb[:]
                if kb % 2 == 0:
                    nc.scalar.mul(dst, xs[:], sh_col[:])
                else:
                    nc.vector.tensor_scalar(
                        dst, xs[:], sh_col[:], None, mybir.AluOpType.mult)

            def load_x_group(mc, inject=None):
                xg = ({}, {})
                for kb in range(KO):
                    if inject and kb in inject:
                        quant_half(*inject[kb])
                    emit_x_chunk(mc, kb, xg)
                return xg

            def chain(nb, mc, xg):
                tiles8, tilesb = xg
                ps = psum_pool.tile([128, M_CHUNK], F32, tag="ps",
                                    name=f"ps{nb}_{mc}")
                for j in range(N_PAIR):
                    nc.tensor.matmul(
                        ps[:], lhsT=qt8[nb][:, 2 * j:2 * j + 2, :],
                        rhs=tiles8[j][:],
                        start=(j == 0), stop=False,
                        perf_mode=mybir.MatmulPerfMode.DoubleRow)
                for i, kb in enumerate(range(KF, KO)):
                    nc.tensor.matmul(
                        ps[:], lhsT=qtb[nb][:, i, :], rhs=tilesb[kb][:],
                        start=(KF == 0 and i == 0), stop=(kb == KO - 1))
                ob = out_pool.tile([128, M_CHUNK], F32, tag="outp",
                                   name=f"ob{nb}_{mc}")
                nc.scalar.copy(ob[:], ps[:])
                nc.scalar.dma_start(
                    outT[128 * nb:128 * (nb + 1),
                         M_CHUNK * mc:M_CHUNK * (mc + 1)], ob[:])

            quant_half(0, 0)
            xg0 = load_x_group(0, inject={
                2: (0, 1), 5: (1, 0), 8: (1, 1), 12: (2, 0), 16: (2, 1),
                20: (3, 0), 24: (3, 1), 28: (4, 0)})
            xg1 = load_x_group(1, inject={
                0: (4, 1), 4: (5, 0), 8: (5, 1), 12: (6, 0), 16: (6, 1),
                20: (7, 0), 24: (7, 1), 28: (8, 0)})
            for nb in range(8):
                if nb < 7:
                    quant_half(nb + 8, 1)
                    quant_half(nb + 9, 0)
                chain(nb, 0, xg0)
                chain(nb, 1, xg1)
            quant_half(15, 1)
            for nb in range(8, N_NB):
                chain(nb, 0, xg0)
            xg2 = load_x_group(2)
            for nb in range(8, N_NB):
                chain(nb, 1, xg1)
            xg3 = load_x_group(3)
            for nb in range(N_NB):
                chain(nb, 2, xg2)
            for nb in range(N_NB):
                chain(nb, 3, xg3)

    nc.compile()
    return nc


def kernel(x, weight, bias):
    global LAST_RESULTS
    x = np.asarray(x, dtype=np.float32)
    weight = np.ascontiguousarray(np.asarray(weight, dtype=np.float32))
    bias = np.ascontiguousarray(np.asarray(bias, dtype=np.float32))

    if "nc_scale" not in _CACHE:
        _CACHE["nc_scale"] = _build_scale()
        _CACHE["nc_main"] = _build_main()
    nc_scale, nc_main = _CACHE["nc_scale"], _CACHE["nc_main"]

    trace = bool(int(os.environ.get("KERNEL_TRACE", "0")))
    kw = {"trace": True, "trace_cores": [0]} if trace else {}

    in_a = [{"wredN": weight[WRED * c:WRED * (c + 1)]}
            for c in range(N_CORES)]
    res_a = run_bass_kernel_spmd(nc_scale, in_a, list(range(N_CORES)), **kw)
    partials = np.array(
        [res_a.results[c]["partial"][0, 0] for c in range(N_CORES)],
        dtype=np.float32)

    xr = x.reshape(M_ALL, K)
    in_b = []
    for c in range(N_CORES):
        i, j = c // F_GRP, c % F_GRP
        w_sh = weight[N_SH * j:N_SH * (j + 1)]
        wt5 = np.ascontiguousarray(
            w_sh.reshape(N_NB, 128, KO, 128).transpose(0, 3, 2, 1))
        in_b.append({
            "xt_sh": np.ascontiguousarray(xr[M_SH * i:M_SH * (i + 1)].T),
            "wt5": wt5,
            "partials": partials,
        })
    res_b = run_bass_kernel_spmd(nc_main, in_b, list(range(N_CORES)), **kw)
    LAST_RESULTS = (res_a, res_b)

    out = np.empty((M_ALL, N_OUT), dtype=np.float32)
    for c in range(N_CORES):
        i, j = c // F_GRP, c % F_GRP
        out[M_SH * i:M_SH * (i + 1), N_SH * j:N_SH * (j + 1)] = \
            res_b.results[c]["outT"].T
    if bias.any():
        out += bias[None, :]
    return out.reshape(B, S, N_OUT)
